# revision 1
# baseline (speedup 1.0000x reference)
"""Trainium2 Bass kernel for an 8-expert top-2 MoE layer (B=4, T=2048, C=1024,
F=4096), expert-parallel across 8 NeuronCores.

Strategy
--------
The reference module is a *dense* MoE: it runs every expert's FFN on every
token, then combines with top-2 gate weights — so 6 of 8 expert outputs per
token are multiplied by zero.  The output only depends on each token's top-2
experts, so we route: the host computes the (tiny) gate in fp32, assigns each
token to its two experts, and the device computes each expert's FFN over just
the tokens routed to it.  The host then scatter-adds the gate-weighted
per-expert outputs.

The gate MUST be computed in fp32: the smallest 2nd-vs-3rd expert logit margin
over the 8192 tokens is ~3.6e-5, and a bf16 gate flips the selected expert set
for ~17 tokens, each flip producing an O(1) relative error at that token.  The
fp32 host gate matches the reference selection with a ~20x margin.

Load balancing: expert token counts vary (~1930..2180), and an SPMD program
pads every core to the busiest expert.  We instead pair a big expert with a
small one (sorted largest<->smallest) and split each pair's FFN across two
cores along the F axis: core 2p+h runs BOTH experts of pair p over F-half h.
Per-core work becomes (n_big + n_small)/2 full-F-equivalents, i.e. the pair
average instead of the global max.  The two cores' partial outputs (each a
full [n, C] sum over its F-half; b2 is pre-halved on the host so the halves
sum to one b2) are added on the host during the scatter.

On-device math per core (pair p, F-half h), for each expert e in the pair:
    hT[f, t]   = sum_c W1[c, f] * xT[c, t]        (PE, bf16 inputs, fp32 acc)
    hT         = gelu_erf(hT + b1[f])             (ScalarE, fused bias)
    out[t, cc] = sum_{f in half} h[t, f] * W2[f, cc]   (PE, bf16 h, fp32 acc)
    out        = out + b2/2                       (VectorE, fp32)
Computing h in transposed form (tokens in the free dim) is what lets the
second matmul contract over F without any on-device transpose.
"""

import os

import numpy as np
import ml_dtypes

import concourse.bass as bass
import concourse.mybir as mybir
import concourse.tile as tile
from concourse import bacc
from concourse.bass_utils import run_bass_kernel_spmd

C = 1024
F = 4096
FH = F // 2  # per-core F half
E = 8
K = 2
N_CORES = 8
CHUNK = 512

BF16 = mybir.dt.bfloat16
F32 = mybir.dt.float32


def build_nc(chunks_a: list[int], chunks_b: list[int]) -> bass.Bass:
    """Bass program: two experts' FFNs (F-half depth) over their token chunks.

    chunks_a/chunks_b: per-chunk token counts for expert slot A / B,
    each 0 < ch <= 512.
    """
    nta, ntb = sum(chunks_a), sum(chunks_b)
    assert all(0 < ch <= 512 for ch in chunks_a + chunks_b)
    nc = bacc.Bacc(None)

    # inputs: token stream and weights for expert slots A and B
    xta = nc.dram_tensor("xta", [C, nta], BF16, kind="ExternalInput")
    xtb = nc.dram_tensor("xtb", [C, ntb], BF16, kind="ExternalInput")
    w1a = nc.dram_tensor("w1a", [C, FH], BF16, kind="ExternalInput")
    w1b = nc.dram_tensor("w1b", [C, FH], BF16, kind="ExternalInput")
    w2a = nc.dram_tensor("w2a", [FH, C], BF16, kind="ExternalInput")
    w2b = nc.dram_tensor("w2b", [FH, C], BF16, kind="ExternalInput")
    # b1t[s][p, j] = b1[slot s][(j*128)+p] for this core's F-half (j: f-tile)
    b1t = nc.dram_tensor("b1t", [2, 128, FH // 128], F32, kind="ExternalInput")
    # b2h[s] = b2[slot s] / 2, broadcast over partitions on device
    b2h = nc.dram_tensor("b2h", [2, C], F32, kind="ExternalInput")
    outa = nc.dram_tensor("outa", [nta, C], F32, kind="ExternalOutput")
    outb = nc.dram_tensor("outb", [ntb, C], F32, kind="ExternalOutput")

    n_ct = C // 128  # 8 contraction tiles for x @ W1
    n_ft = FH // 128  # 16 F tiles per half
    n_cc = C // 512  # 2 output column chunks

    with tile.TileContext(nc) as tc:
        with (
            tc.tile_pool(name="wpool", bufs=1) as wpool,
            tc.tile_pool(name="xpool", bufs=3) as xpool,
            tc.tile_pool(name="hpool", bufs=n_ft + 2) as hpool,
            tc.tile_pool(name="opool", bufs=4) as opool,
            tc.tile_pool(name="phpool", bufs=4, space="PSUM") as phpool,
            tc.tile_pool(name="popool", bufs=4, space="PSUM") as popool,
        ):
            # DMA issue order: biases first (tiny; the first gelu blocks PSUM
            # recycling on b1), chunk-0 activations, W1a in quarters (f-tile
            # order), then W2a / slot-B tensors which are needed later.
            b1_sb = wpool.tile([128, 2, n_ft], F32, name="b1sb", tag="b1sb")
            nc.sync.dma_start(out=b1_sb[:, 0, :], in_=b1t[0])
            nc.sync.dma_start(out=b1_sb[:, 1, :], in_=b1t[1])
            b2_sb = wpool.tile([128, 2, C], F32, name="b2sb", tag="b2sb")
            for s in range(2):
                nc.sync.dma_start(
                    out=b2_sb[:, s, :], in_=b2h[s : s + 1, :].to_broadcast([128, C])
                )

            xts0 = []
            for c in range(n_ct):
                t = xpool.tile([128, chunks_a[0]], BF16, name=f"xta_0_{c}", tag=f"xt{c}")
                nc.sync.dma_start(out=t, in_=xta[c * 128 : (c + 1) * 128, : chunks_a[0]])
                xts0.append(t)

            w1_sb = {
                s: [
                    wpool.tile([128, FH], BF16, name=f"w1sb{s}_{c}", tag=f"w1sb{s}_{c}")
                    for c in range(n_ct)
                ]
                for s in range(2)
            }
            w2_sb = {
                s: [
                    wpool.tile([128, C], BF16, name=f"w2sb{s}_{f}", tag=f"w2sb{s}_{f}")
                    for f in range(n_ft)
                ]
                for s in range(2)
            }
            # slot-A weights up front: W1a in quarters (f-tile order), then W2a
            for quarter in range(4):
                fs = slice(quarter * (FH // 4), (quarter + 1) * (FH // 4))
                for c in range(n_ct):
                    nc.sync.dma_start(
                        out=w1_sb[0][c][:, fs], in_=w1a[c * 128 : (c + 1) * 128, fs]
                    )
            for f in range(n_ft):
                nc.sync.dma_start(out=w2_sb[0][f], in_=w2a[f * 128 : (f + 1) * 128, :])

            # slot-B weight loads, spread between slot-A chunks so they don't
            # starve the slot-A activation streams in the DMA queues.
            deferred_loads = [
                [
                    lambda c=c: nc.sync.dma_start(
                        out=w1_sb[1][c], in_=w1b[c * 128 : (c + 1) * 128, :]
                    )
                    for c in range(n_ct)
                ],
                [
                    lambda f=f: nc.sync.dma_start(
                        out=w2_sb[1][f], in_=w2b[f * 128 : (f + 1) * 128, :]
                    )
                    for f in range(n_ft // 2)
                ],
                [
                    lambda f=f: nc.sync.dma_start(
                        out=w2_sb[1][f], in_=w2b[f * 128 : (f + 1) * 128, :]
                    )
                    for f in range(n_ft // 2, n_ft)
                ],
            ]

            def run_slot(s, xtd, outd, chunks, first_xts):
                tok0 = 0
                for tk, ch in enumerate(chunks):
                    if first_xts is not None and tk == 0:
                        xts = first_xts
                    else:
                        xts = []
                        for c in range(n_ct):
                            t = xpool.tile(
                                [128, ch], BF16, name=f"xt{s}_{tk}_{c}", tag=f"xt{c}"
                            )
                            nc.sync.dma_start(
                                out=t,
                                in_=xtd[c * 128 : (c + 1) * 128, tok0 : tok0 + ch],
                            )
                            xts.append(t)
                    if s == 0 and tk >= 1 and deferred_loads:
                        for emit in deferred_loads.pop(0):
                            emit()

                    hts = []
                    for f in range(n_ft):
                        ph = phpool.tile([128, ch], F32, name=f"ph{s}_{tk}_{f}", tag="ph")
                        for c in range(n_ct):
                            nc.tensor.matmul(
                                ph,
                                lhsT=w1_sb[s][c][:, f * 128 : (f + 1) * 128],
                                rhs=xts[c],
                                start=(c == 0),
                                stop=(c == n_ct - 1),
                            )
                        ht = hpool.tile([128, ch], BF16, name=f"ht{s}_{tk}_{f}", tag="ht")
                        nc.scalar.activation(
                            out=ht,
                            in_=ph,
                            func=mybir.ActivationFunctionType.Gelu,
                            bias=b1_sb[:, s, f : f + 1],
                            scale=1.0,
                        )
                        hts.append(ht)

                    for tt in range((ch + 127) // 128):
                        tw = min(128, ch - tt * 128)
                        for cc in range(n_cc):
                            po = popool.tile(
                                [128, 512], F32, name=f"po{s}_{tk}_{tt}_{cc}", tag="po"
                            )
                            for f in range(n_ft):
                                nc.tensor.matmul(
                                    po[:tw, :],
                                    lhsT=hts[f][:, tt * 128 : tt * 128 + tw],
                                    rhs=w2_sb[s][f][:, cc * 512 : (cc + 1) * 512],
                                    start=(f == 0),
                                    stop=(f == n_ft - 1),
                                )
                            ot = opool.tile(
                                [128, 512], F32, name=f"ot{s}_{tk}_{tt}_{cc}", tag="ot"
                            )
                            nc.vector.tensor_add(
                                ot[:tw, :],
                                po[:tw, :],
                                b2_sb[:tw, s, cc * 512 : (cc + 1) * 512],
                            )
                            r0 = tok0 + tt * 128
                            nc.sync.dma_start(
                                out=outd[r0 : r0 + tw, cc * 512 : (cc + 1) * 512],
                                in_=ot[:tw, :],
                            )
                    tok0 += ch

            run_slot(0, xta, outa, chunks_a, xts0)
            while deferred_loads:  # in case slot A had very few chunks
                for emit in deferred_loads.pop(0):
                    emit()
            run_slot(1, xtb, outb, chunks_b, None)
    nc.finalize()
    return nc


def pick_chunks(n: int) -> list[int]:
    """[512]*a + [exact tail] — matmul N needs no alignment."""
    n512 = n // 512
    rem = n - n512 * 512
    chunks = [512] * n512
    if rem > 0:
        chunks.append(rem)
    if not chunks:
        chunks = [1]
    return chunks


def _route(x2d: np.ndarray, Wg: np.ndarray):
    """fp32 gate identical in selection to the reference; returns per-expert
    token indices and renormalized top-2 weights."""
    logits = x2d @ Wg  # fp32 BLAS
    order = np.argsort(-logits, axis=1, kind="stable")
    top2 = order[:, :K]  # [N, 2]
    m = logits.max(axis=1, keepdims=True)
    p = np.exp(logits - m, dtype=np.float32)
    p /= p.sum(axis=1, keepdims=True)
    tw = np.take_along_axis(p, top2, axis=1)
    tw /= tw.sum(axis=1, keepdims=True)  # [N, 2] renormalized
    idxs, ws = [], []
    for e in range(E):
        sel = top2 == e  # [N, 2] bool, at most one True per row
        rows = np.where(sel.any(axis=1))[0]
        idxs.append(rows)
        ws.append(tw[rows][sel[rows]])
    return idxs, ws


_LAST_RESULTS = {}  # stash for test harness introspection (exec time etc.)


def kernel(**inputs: np.ndarray) -> np.ndarray:
    x = np.asarray(inputs["x"], dtype=np.float32)
    Wg = np.asarray(inputs["Wg"], dtype=np.float32)
    W1 = np.asarray(inputs["W1"], dtype=np.float32)
    b1 = np.asarray(inputs["b1"], dtype=np.float32)
    W2 = np.asarray(inputs["W2"], dtype=np.float32)
    b2 = np.asarray(inputs["b2"], dtype=np.float32)

    B, T, Cx = x.shape
    assert Cx == C
    x2d = np.ascontiguousarray(x.reshape(-1, C))
    n_tok_total = x2d.shape[0]

    idxs, ws = _route(x2d, Wg)
    counts = np.array([len(i) for i in idxs])

    # Pair the largest expert with the smallest, 2nd largest with 2nd
    # smallest, etc.  Pair p runs on cores 2p (F-half 0) and 2p+1 (F-half 1).
    order = np.argsort(-counts, kind="stable")
    pairs = [(int(order[p]), int(order[E - 1 - p])) for p in range(E // 2)]
    nta = max(counts[a] for a, _ in pairs)
    ntb = max(counts[b] for _, b in pairs)
    chunks_a = pick_chunks(int(nta))
    chunks_b = pick_chunks(int(ntb))
    nta, ntb = sum(chunks_a), sum(chunks_b)

    w1h = W1.astype(ml_dtypes.bfloat16)  # [E, C, F]
    w2h = W2.astype(ml_dtypes.bfloat16)  # [E, F, C]

    def xt_for(e, ntok):
        xe = np.zeros((ntok, C), dtype=np.float32)
        xe[: counts[e]] = x2d[idxs[e]]
        return np.ascontiguousarray(xe.T).astype(ml_dtypes.bfloat16)

    xt_cache = {}
    for a, b_ in pairs:
        xt_cache[a] = xt_for(a, nta)
        xt_cache[b_] = xt_for(b_, ntb)

    in_maps = []
    for core in range(N_CORES):
        p, h = divmod(core, 2)
        ea, eb = pairs[p]
        fsl = slice(h * FH, (h + 1) * FH)
        b1t = np.stack(
            [
                np.ascontiguousarray(b1[ea][fsl].reshape(FH // 128, 128).T),
                np.ascontiguousarray(b1[eb][fsl].reshape(FH // 128, 128).T),
            ]
        ).astype(np.float32)
        in_maps.append(
            {
                "xta": xt_cache[ea],
                "xtb": xt_cache[eb],
                "w1a": np.ascontiguousarray(w1h[ea][:, fsl]),
                "w1b": np.ascontiguousarray(w1h[eb][:, fsl]),
                "w2a": np.ascontiguousarray(w2h[ea][fsl, :]),
                "w2b": np.ascontiguousarray(w2h[eb][fsl, :]),
                "b1t": b1t,
                "b2h": np.stack([b2[ea], b2[eb]]).astype(np.float32) * 0.5,
            }
        )

    nc = build_nc(chunks_a, chunks_b)
    trace = os.environ.get("KERNEL_TRACE", "") == "1"
    res = run_bass_kernel_spmd(
        nc, in_maps, core_ids=list(range(N_CORES)), trace=trace
    )
    _LAST_RESULTS["bass_results"] = res
    if trace and res.exec_time_ns is not None:
        print(f"[kernel] HW exec time: {res.exec_time_ns} ns")

    out = np.zeros((n_tok_total, C), dtype=np.float32)
    for p, (ea, eb) in enumerate(pairs):
        for e, key in ((ea, "outa"), (eb, "outb")):
            n_e = counts[e]
            oe = (
                np.asarray(res.results[2 * p][key])[:n_e]
                + np.asarray(res.results[2 * p + 1][key])[:n_e]
            )
            out[idxs[e]] += ws[e][:, None] * oe
    return out.reshape(B, T, C)



# revision 4
# speedup vs baseline: 1.0247x; 1.0247x over previous
"""Trainium2 Bass kernel for an 8-expert top-2 MoE layer (B=4, T=2048, C=1024,
F=4096), expert-parallel across 8 NeuronCores.

Strategy
--------
The reference module is a *dense* MoE: it runs every expert's FFN on every
token, then combines with top-2 gate weights — so 6 of 8 expert outputs per
token are multiplied by zero.  The output only depends on each token's top-2
experts, so we route: the host computes the (tiny) gate in fp32, assigns each
token to its two experts, and the device computes each expert's FFN over just
the tokens routed to it.  The host then scatter-adds the gate-weighted
per-expert outputs.

The gate MUST be computed in fp32: the smallest 2nd-vs-3rd expert logit margin
over the 8192 tokens is ~3.6e-5, and a bf16 gate flips the selected expert set
for ~17 tokens, each flip producing an O(1) relative error at that token.

Load balancing: expert token counts vary (~1930..2180), and an SPMD program
pads every core to the busiest expert.  We pair a big expert with a small one
(sorted largest<->smallest) and split each pair's FFN across two cores along
the F axis: core 2p+h runs BOTH experts of pair p over F-half h.  Per-core
work becomes c1+c5 tokens (~4204) instead of 2*c1.

On-device math per core (pair p, F-half h), for each expert e in the pair:
    hT[f, t]   = sum_c W1[c, f] * xT[c, t]        (PE, bf16, fp32 acc)
    hT         = gelu_erf(hT + b1[f])             (ScalarE, fused bias)
    outT[c, t] = sum_{f in half} W2[f, c] * hT[f, t]   (PE, W2 stationary)
    outT       = outT + b2/2, cast bf16           (ScalarE Copy w/ bias)
L2 keeps W2 as the stationary operand and hT as the moving operand, so the
matmul's free dim is the *exact* chunk token count (no ceil-to-128 tile waste)
and the output lands transposed [C, ntok]; the host transposes back.
L2 of chunk k is issued after L1 of chunk k+1 (software pipelining): the
first L1 chunk's matmuls hide the W2 weight DMA, and L2 never waits on the
last gelu of its own chunk.
"""

import os

import numpy as np
import ml_dtypes

import concourse.bass as bass
import concourse.mybir as mybir
import concourse.tile as tile
from concourse import bacc
from concourse.bass_utils import run_bass_kernel_spmd

C = 1024
F = 4096
FH = F // 2  # per-core F half
E = 8
K = 2
N_CORES = 8
CHUNK = 512

BF16 = mybir.dt.bfloat16
F32 = mybir.dt.float32


def build_nc(chunks_a: list[int], chunks_b: list[int]) -> bass.Bass:
    """Bass program: two experts' FFNs (F-half depth) over their token chunks.

    chunks_a/chunks_b: per-chunk token counts for expert slot A / B,
    each 0 < ch <= 512.
    """
    nta, ntb = sum(chunks_a), sum(chunks_b)
    assert all(0 < ch <= 512 for ch in chunks_a + chunks_b)
    nc = bacc.Bacc(None)

    # inputs: token stream and weights for expert slots A and B
    xta = nc.dram_tensor("xta", [C, nta], BF16, kind="ExternalInput")
    xtb = nc.dram_tensor("xtb", [C, ntb], BF16, kind="ExternalInput")
    w1a = nc.dram_tensor("w1a", [C, FH], BF16, kind="ExternalInput")
    w1b = nc.dram_tensor("w1b", [C, FH], BF16, kind="ExternalInput")
    w2a = nc.dram_tensor("w2a", [FH, C], BF16, kind="ExternalInput")
    w2b = nc.dram_tensor("w2b", [FH, C], BF16, kind="ExternalInput")
    # b1t[s][p, j] = b1[slot s][(j*128)+p] for this core's F-half (j: f-tile)
    b1t = nc.dram_tensor("b1t", [2, 128, FH // 128], F32, kind="ExternalInput")
    # b2t[s][p, j] = b2[slot s][(j*128)+p] / 2 (j: output c-tile)
    b2t = nc.dram_tensor("b2t", [2, 128, C // 128], F32, kind="ExternalInput")
    # transposed bf16 outputs; host adds the two F-half partials and transposes
    outa = nc.dram_tensor("outa", [C, nta], BF16, kind="ExternalOutput")
    outb = nc.dram_tensor("outb", [C, ntb], BF16, kind="ExternalOutput")

    n_ct = C // 128  # 8 contraction tiles for x @ W1
    n_ft = FH // 128  # 16 F tiles per half
    n_cg = C // 128  # 8 output c-tile groups for L2

    with tile.TileContext(nc) as tc:
        with (
            tc.tile_pool(name="wpool", bufs=1) as wpool,
            tc.tile_pool(name="xpool", bufs=2) as xpool,
            tc.tile_pool(name="hpool", bufs=2 * n_ft + 2) as hpool,
            tc.tile_pool(name="opool", bufs=4) as opool,
            tc.tile_pool(name="phpool", bufs=4, space="PSUM") as phpool,
            tc.tile_pool(name="popool", bufs=4, space="PSUM") as popool,
        ):
            b1_sb = wpool.tile([128, 2, n_ft], F32, name="b1sb", tag="b1sb")
            b2_sb = wpool.tile([128, 2, n_cg], F32, name="b2sb", tag="b2sb")
            for s in range(2):
                nc.sync.dma_start(out=b1_sb[:, s, :], in_=b1t[s])
                nc.sync.dma_start(out=b2_sb[:, s, :], in_=b2t[s])

            w1_sb = {
                s: [
                    wpool.tile([128, FH], BF16, name=f"w1sb{s}_{c}", tag=f"w1sb{s}_{c}")
                    for c in range(n_ct)
                ]
                for s in range(2)
            }
            w2_sb = {
                s: [
                    wpool.tile([128, C], BF16, name=f"w2sb{s}_{f}", tag=f"w2sb{s}_{f}")
                    for f in range(n_ft)
                ]
                for s in range(2)
            }

            # x is DMA'd in chunk-PAIRS (2KB lines) — map chunk idx -> (pair
            # tile tag-gen, offset).  Pairs: (0,1), (2,3), ... possibly a lone
            # tail.
            def chunk_pairs(chunks):
                pairs = []
                i = 0
                while i < len(chunks):
                    if i + 1 < len(chunks):
                        pairs.append((i, chunks[i] + chunks[i + 1]))
                        i += 2
                    else:
                        pairs.append((i, chunks[i]))
                        i += 1
                return pairs

            # ---- startup-critical DMA order ----
            # interleave x-chunk0/1 pair with W1a f-half-0 per c-tile (the
            # first L1 f-tile group needs exactly these), then W1a f-half-1,
            # then W2a (hidden under L1 chunk 0 thanks to the L2 pipeline
            # lag), then the rest.
            pairs_a = chunk_pairs(chunks_a)
            pairs_b = chunk_pairs(chunks_b)
            xa_tiles = {}
            xb_tiles = {}

            p0_start, p0_w = pairs_a[0]
            xa0 = [
                xpool.tile([128, p0_w], BF16, name=f"xta_p0_{c}", tag=f"xt{c}")
                for c in range(n_ct)
            ]
            xa_tiles[0] = (xa0, 0)
            if len(chunks_a) > 1:
                xa_tiles[1] = (xa0, chunks_a[0])
            for c in range(n_ct):
                nc.sync.dma_start(out=xa0[c], in_=xta[c * 128 : (c + 1) * 128, :p0_w])
                nc.sync.dma_start(
                    out=w1_sb[0][c][:, : FH // 2],
                    in_=w1a[c * 128 : (c + 1) * 128, : FH // 2],
                )
            for c in range(n_ct):
                nc.sync.dma_start(
                    out=w1_sb[0][c][:, FH // 2 :],
                    in_=w1a[c * 128 : (c + 1) * 128, FH // 2 :],
                )
            for f in range(n_ft):
                nc.sync.dma_start(out=w2_sb[0][f], in_=w2a[f * 128 : (f + 1) * 128, :])

            # deferred loads, emitted between later chunks so they don't
            # starve the slot-A streams
            def w1_load(s, c, h):
                w1d = w1b if s else w1a
                return lambda: nc.sync.dma_start(
                    out=w1_sb[s][c][:, h * (FH // 2) : (h + 1) * (FH // 2)],
                    in_=w1d[c * 128 : (c + 1) * 128, h * (FH // 2) : (h + 1) * (FH // 2)],
                )

            def w2_load(s, f):
                w2d = w2b if s else w2a
                return lambda: nc.sync.dma_start(
                    out=w2_sb[s][f], in_=w2d[f * 128 : (f + 1) * 128, :]
                )

            deferred_loads = [
                [w1_load(1, c, h) for c in range(n_ct) for h in range(2)],
                [w2_load(1, f) for f in range(n_ft // 2)],
                [w2_load(1, f) for f in range(n_ft // 2, n_ft)],
            ]

            def emit_x_pair(s, pi):
                pairs, xtd, tiles = (
                    (pairs_a, xta, xa_tiles) if s == 0 else (pairs_b, xtb, xb_tiles)
                )
                chunks = chunks_a if s == 0 else chunks_b
                ci, w = pairs[pi]
                tok0 = sum(chunks[:ci])
                ts = [
                    xpool.tile([128, w], BF16, name=f"xt{s}_p{pi}_{c}", tag=f"xt{c}")
                    for c in range(n_ct)
                ]
                for c in range(n_ct):
                    nc.sync.dma_start(
                        out=ts[c], in_=xtd[c * 128 : (c + 1) * 128, tok0 : tok0 + w]
                    )
                tiles[ci] = (ts, 0)
                if ci + 1 < len(chunks) and w > chunks[ci]:
                    tiles[ci + 1] = (ts, chunks[ci])

            # software-pipelined chunk schedule: L1(k); L2(k-1)
            sched = []  # (slot, chunk_idx, tok0, ch)
            for s, chunks in ((0, chunks_a), (1, chunks_b)):
                tok0 = 0
                for ci, ch in enumerate(chunks):
                    sched.append((s, ci, tok0, ch))
                    tok0 += ch

            def do_l1(item):
                s, ci, tok0, ch = item
                tiles = xa_tiles if s == 0 else xb_tiles
                xts, off = tiles[ci]
                hts = []
                for f in range(n_ft):
                    ph = phpool.tile([128, ch], F32, name=f"ph{s}_{ci}_{f}", tag="ph")
                    for c in range(n_ct):
                        nc.tensor.matmul(
                            ph,
                            lhsT=w1_sb[s][c][:, f * 128 : (f + 1) * 128],
                            rhs=xts[c][:, off : off + ch],
                            start=(c == 0),
                            stop=(c == n_ct - 1),
                        )
                    ht = hpool.tile([128, ch], BF16, name=f"ht{s}_{ci}_{f}", tag="ht")
                    nc.scalar.activation(
                        out=ht,
                        in_=ph,
                        func=mybir.ActivationFunctionType.Gelu,
                        bias=b1_sb[:, s, f : f + 1],
                        scale=1.0,
                    )
                    hts.append(ht)
                return hts

            def do_l2(item, hts):
                s, ci, tok0, ch = item
                outd = outa if s == 0 else outb
                for cg in range(n_cg):
                    po = popool.tile([128, 512], F32, name=f"po{s}_{ci}_{cg}", tag="po")
                    for f in range(n_ft):
                        nc.tensor.matmul(
                            po[:, :ch],
                            lhsT=w2_sb[s][f][:, cg * 128 : (cg + 1) * 128],
                            rhs=hts[f],
                            start=(f == 0),
                            stop=(f == n_ft - 1),
                        )
                    ot = opool.tile([128, 512], BF16, name=f"ot{s}_{ci}_{cg}", tag="ot")
                    nc.scalar.activation(
                        out=ot[:, :ch],
                        in_=po[:, :ch],
                        func=mybir.ActivationFunctionType.Identity,
                        bias=b2_sb[:, s, cg : cg + 1],
                        scale=1.0,
                    )
                    nc.sync.dma_start(
                        out=outd[cg * 128 : (cg + 1) * 128, tok0 : tok0 + ch],
                        in_=ot[:, :ch],
                    )

            prev = None
            prev_hts = None
            for k, item in enumerate(sched):
                s, ci, tok0, ch = item
                pairs_s = pairs_a if s == 0 else pairs_b
                tiles = xa_tiles if s == 0 else xb_tiles
                if ci not in tiles:
                    emit_x_pair(
                        s, next(i for i, (c0, _) in enumerate(pairs_s) if c0 == ci)
                    )
                # on entering a pair's first chunk, prefetch the next pair;
                # on entering slot A's last pair, prefetch slot B's first
                for pi, (c0, _) in enumerate(pairs_s):
                    if c0 == ci and pi + 1 < len(pairs_s):
                        if pairs_s[pi + 1][0] not in tiles:
                            emit_x_pair(s, pi + 1)
                if s == 0 and ci == pairs_a[-1][0] and 0 not in xb_tiles:
                    emit_x_pair(1, 0)
                if k >= 1 and deferred_loads:
                    for emit in deferred_loads.pop(0):
                        emit()
                hts = do_l1(item)
                if prev is not None:
                    do_l2(prev, prev_hts)
                prev, prev_hts = item, hts
            while deferred_loads:
                for emit in deferred_loads.pop(0):
                    emit()
            do_l2(prev, prev_hts)
    nc.finalize()
    return nc


def pick_chunks(n: int) -> list[int]:
    """[512]*a + [exact tail] — matmul N needs no alignment."""
    n512 = n // 512
    rem = n - n512 * 512
    chunks = [512] * n512
    if rem > 0:
        chunks.append(rem)
    if not chunks:
        chunks = [1]
    return chunks


def _route(x2d: np.ndarray, Wg: np.ndarray):
    """fp32 gate identical in selection to the reference; returns per-expert
    token indices and renormalized top-2 weights."""
    logits = x2d @ Wg  # fp32 BLAS
    order = np.argsort(-logits, axis=1, kind="stable")
    top2 = order[:, :K]  # [N, 2]
    m = logits.max(axis=1, keepdims=True)
    p = np.exp(logits - m, dtype=np.float32)
    p /= p.sum(axis=1, keepdims=True)
    tw = np.take_along_axis(p, top2, axis=1)
    tw /= tw.sum(axis=1, keepdims=True)  # [N, 2] renormalized
    idxs, ws = [], []
    for e in range(E):
        sel = top2 == e  # [N, 2] bool, at most one True per row
        rows = np.where(sel.any(axis=1))[0]
        idxs.append(rows)
        ws.append(tw[rows][sel[rows]])
    return idxs, ws


_LAST_RESULTS = {}  # stash for test harness introspection (exec time etc.)


def kernel(**inputs: np.ndarray) -> np.ndarray:
    x = np.asarray(inputs["x"], dtype=np.float32)
    Wg = np.asarray(inputs["Wg"], dtype=np.float32)
    W1 = np.asarray(inputs["W1"], dtype=np.float32)
    b1 = np.asarray(inputs["b1"], dtype=np.float32)
    W2 = np.asarray(inputs["W2"], dtype=np.float32)
    b2 = np.asarray(inputs["b2"], dtype=np.float32)

    B, T, Cx = x.shape
    assert Cx == C
    x2d = np.ascontiguousarray(x.reshape(-1, C))
    n_tok_total = x2d.shape[0]

    idxs, ws = _route(x2d, Wg)
    counts = np.array([len(i) for i in idxs])

    # Pair the largest expert with the smallest, 2nd largest with 2nd
    # smallest, etc.  Pair p runs on cores 2p (F-half 0) and 2p+1 (F-half 1).
    order = np.argsort(-counts, kind="stable")
    pairs = [(int(order[p]), int(order[E - 1 - p])) for p in range(E // 2)]
    nta = max(counts[a] for a, _ in pairs)
    ntb = max(counts[b] for _, b in pairs)
    chunks_a = pick_chunks(int(nta))
    chunks_b = pick_chunks(int(ntb))
    nta, ntb = sum(chunks_a), sum(chunks_b)

    w1h = W1.astype(ml_dtypes.bfloat16)  # [E, C, F]
    w2h = W2.astype(ml_dtypes.bfloat16)  # [E, F, C]

    def xt_for(e, ntok):
        xe = np.zeros((ntok, C), dtype=np.float32)
        xe[: counts[e]] = x2d[idxs[e]]
        return np.ascontiguousarray(xe.T).astype(ml_dtypes.bfloat16)

    xt_cache = {}
    for a, b_ in pairs:
        xt_cache[a] = xt_for(a, nta)
        xt_cache[b_] = xt_for(b_, ntb)

    in_maps = []
    for core in range(N_CORES):
        p, h = divmod(core, 2)
        ea, eb = pairs[p]
        fsl = slice(h * FH, (h + 1) * FH)
        b1t = np.stack(
            [
                np.ascontiguousarray(b1[ea][fsl].reshape(FH // 128, 128).T),
                np.ascontiguousarray(b1[eb][fsl].reshape(FH // 128, 128).T),
            ]
        ).astype(np.float32)
        b2t = np.stack(
            [
                np.ascontiguousarray(b2[ea].reshape(C // 128, 128).T),
                np.ascontiguousarray(b2[eb].reshape(C // 128, 128).T),
            ]
        ).astype(np.float32) * 0.5
        in_maps.append(
            {
                "xta": xt_cache[ea],
                "xtb": xt_cache[eb],
                "w1a": np.ascontiguousarray(w1h[ea][:, fsl]),
                "w1b": np.ascontiguousarray(w1h[eb][:, fsl]),
                "w2a": np.ascontiguousarray(w2h[ea][fsl, :]),
                "w2b": np.ascontiguousarray(w2h[eb][fsl, :]),
                "b1t": b1t,
                "b2t": b2t,
            }
        )

    nc = build_nc(chunks_a, chunks_b)
    trace = os.environ.get("KERNEL_TRACE", "") == "1"
    res = run_bass_kernel_spmd(
        nc, in_maps, core_ids=list(range(N_CORES)), trace=trace
    )
    _LAST_RESULTS["bass_results"] = res
    if trace and res.exec_time_ns is not None:
        print(f"[kernel] HW exec time: {res.exec_time_ns} ns")

    out = np.zeros((n_tok_total, C), dtype=np.float32)
    for p, (ea, eb) in enumerate(pairs):
        for e, key in ((ea, "outa"), (eb, "outb")):
            n_e = counts[e]
            oe = (
                np.asarray(res.results[2 * p][key]).astype(np.float32)
                + np.asarray(res.results[2 * p + 1][key]).astype(np.float32)
            ).T[:n_e]
            out[idxs[e]] += ws[e][:, None] * oe
    return out.reshape(B, T, C)


# revision 5
# speedup vs baseline: 1.0250x; 1.0003x over previous
"""Trainium2 Bass kernel for an 8-expert top-2 MoE layer (B=4, T=2048, C=1024,
F=4096), expert-parallel across 8 NeuronCores.

Strategy
--------
The reference module is a *dense* MoE: it runs every expert's FFN on every
token, then combines with top-2 gate weights — so 6 of 8 expert outputs per
token are multiplied by zero.  The output only depends on each token's top-2
experts, so we route: the host computes the (tiny) gate in fp32, assigns each
token to its two experts, and the device computes each expert's FFN over just
the tokens routed to it.  The host then scatter-adds the gate-weighted
per-expert outputs.

The gate MUST be computed in fp32: the smallest 2nd-vs-3rd expert logit margin
over the 8192 tokens is ~3.6e-5, and a bf16 gate flips the selected expert set
for ~17 tokens, each flip producing an O(1) relative error at that token.

Load balancing: expert token counts vary (~1930..2180), and an SPMD program
pads every core to the busiest expert.  We pair a big expert with a small one
(sorted largest<->smallest) and split each pair's FFN across two cores along
the F axis: core 2p+h runs BOTH experts of pair p over F-half h.

On-device math per core (pair p, F-half h), for each expert e in the pair:
    hT[f, t]   = sum_c W1[c, f] * xT[c, t]        (PE, bf16, fp32 acc)
    hT         = gelu_erf(hT + b1[f])             (ScalarE, fused bias)
    outT[c, t] = sum_{f in half} W2[f, c] * hT[f, t]   (PE, W2 stationary)
    outT       = outT + b2/2, cast bf16           (ScalarE Identity w/ bias)
L2 keeps W2 stationary and hT moving, so the matmul free dim is the *exact*
chunk token count (no ceil-to-128 tile waste) and the output lands transposed;
the host transposes back.  L2 of chunk k is issued after L1 of chunk k+1
(software pipelining): L1 of the first chunk hides the W2 DMA and L2 never
waits on its own chunk's last gelu.

All large tensors use partition-major DRAM layouts ([128, ktiles, free]) so
each one moves in O(1) dma_start calls — DMA *issue* on the sync queue costs
~0.8us each, and with per-tile DMAs the startup was issue-bound, not
bandwidth-bound.
"""

import os

import numpy as np
import ml_dtypes

import concourse.bass as bass
import concourse.mybir as mybir
import concourse.tile as tile
from concourse import bacc
from concourse.bass_utils import run_bass_kernel_spmd

C = 1024
F = 4096
FH = F // 2  # per-core F half
E = 8
K = 2
N_CORES = 8
CHUNK = 512

BF16 = mybir.dt.bfloat16
F32 = mybir.dt.float32


def build_nc(chunks_a: list[int], chunks_b: list[int]) -> bass.Bass:
    """Bass program: two experts' FFNs (F-half depth) over their token chunks."""
    nta, ntb = sum(chunks_a), sum(chunks_b)
    assert all(0 < ch <= 512 for ch in chunks_a + chunks_b)
    nc = bacc.Bacc(None)

    n_ct = C // 128  # 8 contraction tiles for x @ W1
    n_ft = FH // 128  # 16 F tiles per half
    n_cg = C // 128  # 8 output c-tile groups for L2

    # partition-major layouts: [...][p, k_tile, free]
    xta = nc.dram_tensor("xta", [128, n_ct, nta], BF16, kind="ExternalInput")
    xtb = nc.dram_tensor("xtb", [128, n_ct, ntb], BF16, kind="ExternalInput")
    w1a = nc.dram_tensor("w1a", [128, n_ct, FH], BF16, kind="ExternalInput")
    w1b = nc.dram_tensor("w1b", [128, n_ct, FH], BF16, kind="ExternalInput")
    w2a = nc.dram_tensor("w2a", [128, n_ft, C], BF16, kind="ExternalInput")
    w2b = nc.dram_tensor("w2b", [128, n_ft, C], BF16, kind="ExternalInput")
    # bt[s][p, j]: j<16 -> b1[slot s][j*128+p]; j>=16 -> b2[slot s][(j-16)*128+p]/2
    bt = nc.dram_tensor("bt", [2, 128, n_ft + n_cg], F32, kind="ExternalInput")
    outa = nc.dram_tensor("outa", [128, n_cg, nta], BF16, kind="ExternalOutput")
    outb = nc.dram_tensor("outb", [128, n_cg, ntb], BF16, kind="ExternalOutput")

    with tile.TileContext(nc) as tc:
        with (
            tc.tile_pool(name="wpool", bufs=1) as wpool,
            tc.tile_pool(name="xpool", bufs=3) as xpool,
            tc.tile_pool(name="hpool", bufs=2 * n_ft + 2) as hpool,
            tc.tile_pool(name="opool", bufs=2) as opool,
            tc.tile_pool(name="phpool", bufs=4, space="PSUM") as phpool,
            tc.tile_pool(name="popool", bufs=4, space="PSUM") as popool,
        ):
            w1_sb = {
                s: wpool.tile([128, n_ct, FH], BF16, name=f"w1sb{s}", tag=f"w1sb{s}")
                for s in range(2)
            }
            w2_sb = {
                s: wpool.tile([128, n_ft, C], BF16, name=f"w2sb{s}", tag=f"w2sb{s}")
                for s in range(2)
            }
            b_sb = wpool.tile([128, 2, n_ft + n_cg], F32, name="bsb", tag="bsb")

            # global chunk schedule: (slot, chunk_idx_in_slot, tok0, ch)
            sched = []
            for s, chunks in ((0, chunks_a), (1, chunks_b)):
                tok0 = 0
                for ci, ch in enumerate(chunks):
                    sched.append((s, ci, tok0, ch))
                    tok0 += ch

            x_tiles = {}

            def emit_x(k):
                s, ci, tok0, ch = sched[k]
                xtd = xta if s == 0 else xtb
                t = xpool.tile([128, n_ct, ch], BF16, name=f"xt{k}", tag="xt")
                nc.sync.dma_start(out=t, in_=xtd[:, :, tok0 : tok0 + ch])
                x_tiles[k] = t

            # ---- startup-critical DMA order ----
            # x chunk-0 and W1a f-quarter-0 gate the first L1 f-tile group;
            # everything after streams under compute (L2 lags L1 by a chunk,
            # so W2a hides under L1 of chunk 0).
            emit_x(0)
            nc.sync.dma_start(out=w1_sb[0][:, :, : FH // 4], in_=w1a[:, :, : FH // 4])
            for s in range(2):
                nc.sync.dma_start(out=b_sb[:, s, :], in_=bt[s])
            nc.sync.dma_start(out=w1_sb[0][:, :, FH // 4 :], in_=w1a[:, :, FH // 4 :])
            nc.sync.dma_start(out=w2_sb[0], in_=w2a[:, :, :])
            if len(sched) > 1:
                emit_x(1)
            if len(sched) > 2:
                emit_x(2)

            deferred_loads = [
                [lambda: nc.sync.dma_start(out=w1_sb[1], in_=w1b[:, :, :])],
                [lambda: nc.sync.dma_start(out=w2_sb[1], in_=w2b[:, :, :])],
            ]

            def do_l1(k):
                s, ci, tok0, ch = sched[k]
                xt = x_tiles[k]
                hts = []
                for f in range(n_ft):
                    ph = phpool.tile([128, ch], F32, name=f"ph{k}_{f}", tag="ph")
                    for c in range(n_ct):
                        nc.tensor.matmul(
                            ph,
                            lhsT=w1_sb[s][:, c, f * 128 : (f + 1) * 128],
                            rhs=xt[:, c, :],
                            start=(c == 0),
                            stop=(c == n_ct - 1),
                        )
                    ht = hpool.tile([128, ch], BF16, name=f"ht{k}_{f}", tag="ht")
                    nc.scalar.activation(
                        out=ht,
                        in_=ph,
                        func=mybir.ActivationFunctionType.Gelu,
                        bias=b_sb[:, s, f : f + 1],
                        scale=1.0,
                    )
                    hts.append(ht)
                return hts

            def do_l2(k, hts):
                s, ci, tok0, ch = sched[k]
                outd = outa if s == 0 else outb
                ot = opool.tile([128, n_cg, 512], BF16, name=f"ot{k}", tag="ot")
                for cg in range(n_cg):
                    po = popool.tile([128, 512], F32, name=f"po{k}_{cg}", tag="po")
                    for f in range(n_ft):
                        nc.tensor.matmul(
                            po[:, :ch],
                            lhsT=w2_sb[s][:, f, cg * 128 : (cg + 1) * 128],
                            rhs=hts[f],
                            start=(f == 0),
                            stop=(f == n_ft - 1),
                        )
                    nc.scalar.activation(
                        out=ot[:, cg, :ch],
                        in_=po[:, :ch],
                        func=mybir.ActivationFunctionType.Identity,
                        bias=b_sb[:, s, n_ft + cg : n_ft + cg + 1],
                        scale=1.0,
                    )
                    if cg == n_cg // 2 - 1:
                        nc.sync.dma_start(
                            out=outd[:, : n_cg // 2, tok0 : tok0 + ch],
                            in_=ot[:, : n_cg // 2, :ch],
                        )
                nc.sync.dma_start(
                    out=outd[:, n_cg // 2 :, tok0 : tok0 + ch],
                    in_=ot[:, n_cg // 2 :, :ch],
                )

            prev = None
            prev_hts = None
            for k in range(len(sched)):
                if k + 3 < len(sched):
                    emit_x(k + 3)
                if k >= 1 and deferred_loads:
                    for emit in deferred_loads.pop(0):
                        emit()
                hts = do_l1(k)
                if prev is not None:
                    do_l2(prev, prev_hts)
                prev, prev_hts = k, hts
            do_l2(prev, prev_hts)
    nc.finalize()
    return nc


def pick_chunks(n: int) -> list[int]:
    """[512]*a + [exact tail] — matmul N needs no alignment."""
    n512 = n // 512
    rem = n - n512 * 512
    chunks = [512] * n512
    if rem > 0:
        chunks.append(rem)
    if not chunks:
        chunks = [1]
    return chunks


def _route(x2d: np.ndarray, Wg: np.ndarray):
    """fp32 gate identical in selection to the reference; returns per-expert
    token indices and renormalized top-2 weights."""
    logits = x2d @ Wg  # fp32 BLAS
    order = np.argsort(-logits, axis=1, kind="stable")
    top2 = order[:, :K]  # [N, 2]
    m = logits.max(axis=1, keepdims=True)
    p = np.exp(logits - m, dtype=np.float32)
    p /= p.sum(axis=1, keepdims=True)
    tw = np.take_along_axis(p, top2, axis=1)
    tw /= tw.sum(axis=1, keepdims=True)  # [N, 2] renormalized
    idxs, ws = [], []
    for e in range(E):
        sel = top2 == e  # [N, 2] bool, at most one True per row
        rows = np.where(sel.any(axis=1))[0]
        idxs.append(rows)
        ws.append(tw[rows][sel[rows]])
    return idxs, ws


def _pmajor(a: np.ndarray, ktiles: int) -> np.ndarray:
    """[ktiles*128, free] -> contiguous [128, ktiles, free]."""
    kt, rem = divmod(a.shape[0], 128)
    assert rem == 0 and kt == ktiles
    return np.ascontiguousarray(a.reshape(ktiles, 128, -1).transpose(1, 0, 2))


_LAST_RESULTS = {}  # stash for test harness introspection (exec time etc.)


def kernel(**inputs: np.ndarray) -> np.ndarray:
    x = np.asarray(inputs["x"], dtype=np.float32)
    Wg = np.asarray(inputs["Wg"], dtype=np.float32)
    W1 = np.asarray(inputs["W1"], dtype=np.float32)
    b1 = np.asarray(inputs["b1"], dtype=np.float32)
    W2 = np.asarray(inputs["W2"], dtype=np.float32)
    b2 = np.asarray(inputs["b2"], dtype=np.float32)

    B, T, Cx = x.shape
    assert Cx == C
    x2d = np.ascontiguousarray(x.reshape(-1, C))
    n_tok_total = x2d.shape[0]

    idxs, ws = _route(x2d, Wg)
    counts = np.array([len(i) for i in idxs])

    # Pair the largest expert with the smallest, 2nd largest with 2nd
    # smallest, etc.  Pair p runs on cores 2p (F-half 0) and 2p+1 (F-half 1).
    order = np.argsort(-counts, kind="stable")
    pairs = [(int(order[p]), int(order[E - 1 - p])) for p in range(E // 2)]
    nta = max(counts[a] for a, _ in pairs)
    ntb = max(counts[b] for _, b in pairs)
    chunks_a = pick_chunks(int(nta))
    chunks_b = pick_chunks(int(ntb))
    nta, ntb = sum(chunks_a), sum(chunks_b)

    w1h = W1.astype(ml_dtypes.bfloat16)  # [E, C, F]
    w2h = W2.astype(ml_dtypes.bfloat16)  # [E, F, C]

    def xt_for(e, ntok):
        xe = np.zeros((ntok, C), dtype=np.float32)
        xe[: counts[e]] = x2d[idxs[e]]
        return _pmajor(xe.T.astype(ml_dtypes.bfloat16), C // 128)

    xt_cache = {}
    for a, b_ in pairs:
        xt_cache[a] = xt_for(a, nta)
        xt_cache[b_] = xt_for(b_, ntb)

    in_maps = []
    for core in range(N_CORES):
        p, h = divmod(core, 2)
        ea, eb = pairs[p]
        fsl = slice(h * FH, (h + 1) * FH)
        bt = np.stack(
            [
                np.concatenate(
                    [
                        b1[e][fsl].reshape(FH // 128, 128).T,
                        b2[e].reshape(C // 128, 128).T * 0.5,
                    ],
                    axis=1,
                )
                for e in (ea, eb)
            ]
        ).astype(np.float32)
        in_maps.append(
            {
                "xta": xt_cache[ea],
                "xtb": xt_cache[eb],
                "w1a": _pmajor(np.ascontiguousarray(w1h[ea][:, fsl]), C // 128),
                "w1b": _pmajor(np.ascontiguousarray(w1h[eb][:, fsl]), C // 128),
                "w2a": _pmajor(np.ascontiguousarray(w2h[ea][fsl, :]), FH // 128),
                "w2b": _pmajor(np.ascontiguousarray(w2h[eb][fsl, :]), FH // 128),
                "bt": np.ascontiguousarray(bt),
            }
        )

    nc = build_nc(chunks_a, chunks_b)
    trace = os.environ.get("KERNEL_TRACE", "") == "1"
    res = run_bass_kernel_spmd(
        nc, in_maps, core_ids=list(range(N_CORES)), trace=trace
    )
    _LAST_RESULTS["bass_results"] = res
    if trace and res.exec_time_ns is not None:
        print(f"[kernel] HW exec time: {res.exec_time_ns} ns")

    out = np.zeros((n_tok_total, C), dtype=np.float32)
    for p, (ea, eb) in enumerate(pairs):
        for e, key in ((ea, "outa"), (eb, "outb")):
            n_e = counts[e]
            o0 = np.asarray(res.results[2 * p][key]).astype(np.float32)
            o1 = np.asarray(res.results[2 * p + 1][key]).astype(np.float32)
            oe = (o0 + o1).transpose(1, 0, 2).reshape(C, -1).T[:n_e]
            out[idxs[e]] += ws[e][:, None] * oe
    return out.reshape(B, T, C)


# revision 8
# speedup vs baseline: 1.0349x; 1.0097x over previous
"""Trainium2 Bass kernel for an 8-expert top-2 MoE layer (B=4, T=2048, C=1024,
F=4096), expert-parallel across 8 NeuronCores.

Strategy
--------
The reference module is a *dense* MoE: it runs every expert's FFN on every
token, then combines with top-2 gate weights — so 6 of 8 expert outputs per
token are multiplied by zero.  The output only depends on each token's top-2
experts, so we route: the host computes the (tiny) gate in fp32, assigns each
token to its two experts, and the device computes each expert's FFN over just
the tokens routed to it.  The host then scatter-adds the gate-weighted
per-expert outputs.

The gate MUST be computed in fp32: the smallest 2nd-vs-3rd expert logit margin
over the 8192 tokens is ~3.6e-5, and a bf16 gate flips the selected expert set
for ~17 tokens, each flip producing an O(1) relative error at that token.

Load balancing: expert token counts vary (~1930..2180), and an SPMD program
pads every core to the busiest expert.  We pair a big expert with a small one
(sorted largest<->smallest) and split each pair's FFN across two cores along
the F axis: core 2p+h runs BOTH experts of pair p over F-half h.

On-device math per core (pair p, F-half h), for each expert e in the pair:
    hT[f, t]   = sum_c W1[c, f] * xT[c, t]        (PE, bf16, fp32 acc)
    hT         = gelu_erf(hT + b1[f])             (ScalarE, fused bias)
    outT[c, t] = sum_{f in half} W2[f, c] * hT[f, t]   (PE, W2 stationary)
    outT       = outT + b2/2, cast bf16           (ScalarE Identity w/ bias)
L2 keeps W2 stationary and hT moving, so the matmul free dim is the *exact*
chunk token count (no ceil-to-128 tile waste) and the output lands transposed;
the host transposes back.  L2 of chunk k is issued after L1 of chunk k+1
(software pipelining): L1 of the first chunk hides the W2 DMA and L2 never
waits on its own chunk's last gelu.

All large tensors use partition-major DRAM layouts ([128, ktiles, free]) so
each one moves in O(1) dma_start calls — DMA *issue* on the sync queue costs
~0.8us each, and with per-tile DMAs the startup was issue-bound, not
bandwidth-bound.
"""

import os

import numpy as np
import ml_dtypes

import concourse.bass as bass
import concourse.mybir as mybir
import concourse.tile as tile
from concourse import bacc
from concourse.bass_utils import run_bass_kernel_spmd

C = 1024
F = 4096
FH = F // 2  # per-core F half
E = 8
K = 2
N_CORES = 8
CHUNK = 512

BF16 = mybir.dt.bfloat16
F32 = mybir.dt.float32


def build_nc(chunks_a: list[int], chunks_b: list[int]) -> bass.Bass:
    """Bass program: two experts' FFNs (F-half depth) over their token chunks."""
    nta, ntb = sum(chunks_a), sum(chunks_b)
    assert all(0 < ch <= 512 for ch in chunks_a + chunks_b)
    nc = bacc.Bacc(None)

    n_ct = C // 128  # 8 contraction tiles for x @ W1
    n_ft = FH // 128  # 16 F tiles per half
    n_cg = C // 128  # 8 output c-tile groups for L2

    # partition-major layouts: [...][p, k_tile, free]
    xta = nc.dram_tensor("xta", [128, n_ct, nta], BF16, kind="ExternalInput")
    xtb = nc.dram_tensor("xtb", [128, n_ct, ntb], BF16, kind="ExternalInput")
    w1a = nc.dram_tensor("w1a", [128, n_ct, FH], BF16, kind="ExternalInput")
    w1b = nc.dram_tensor("w1b", [128, n_ct, FH], BF16, kind="ExternalInput")
    w2a = nc.dram_tensor("w2a", [128, n_ft, C], BF16, kind="ExternalInput")
    w2b = nc.dram_tensor("w2b", [128, n_ft, C], BF16, kind="ExternalInput")
    # bt[s][p, j]: j<16 -> b1[slot s][j*128+p]; j>=16 -> b2[slot s][(j-16)*128+p]/2
    bt = nc.dram_tensor("bt", [2, 128, n_ft + n_cg], F32, kind="ExternalInput")
    outa = nc.dram_tensor("outa", [128, n_cg, nta], BF16, kind="ExternalOutput")
    outb = nc.dram_tensor("outb", [128, n_cg, ntb], BF16, kind="ExternalOutput")

    with tile.TileContext(nc) as tc:
        with (
            tc.tile_pool(name="wpool", bufs=1) as wpool,
            tc.tile_pool(name="xpool", bufs=3) as xpool,
            tc.tile_pool(name="hpool", bufs=2 * n_ft + 2) as hpool,
            tc.tile_pool(name="opool", bufs=2) as opool,
            tc.tile_pool(name="phpool", bufs=3, space="PSUM") as phpool,
            tc.tile_pool(name="popool", bufs=4, space="PSUM") as popool,
            tc.tile_pool(name="wupool", bufs=1, space="PSUM") as wupool,
        ):
            # PE warmup: dummy matmuls with no DMA dependency spin the Tensor
            # engine during the initial DMA fill so the clock (HAM pstate) is
            # fully ramped when the first real matmul issues.
            dmy = wpool.tile([128, 512], BF16, name="dmy", tag="dmy")
            nc.vector.memset(dmy, 0.0)
            wups = wupool.tile([128, 512], F32, name="wups", tag="wups")
            for _ in range(5):
                nc.tensor.matmul(wups, lhsT=dmy[:, :128], rhs=dmy, start=True, stop=True)
            w1_sb = {
                s: wpool.tile([128, n_ct, FH], BF16, name=f"w1sb{s}", tag=f"w1sb{s}")
                for s in range(2)
            }
            w2_sb = {
                s: wpool.tile([128, n_ft, C], BF16, name=f"w2sb{s}", tag=f"w2sb{s}")
                for s in range(2)
            }
            b_sb = wpool.tile([128, 2, n_ft + n_cg], F32, name="bsb", tag="bsb")

            # global chunk schedule: (slot, chunk_idx_in_slot, tok0, ch)
            sched = []
            for s, chunks in ((0, chunks_a), (1, chunks_b)):
                tok0 = 0
                for ci, ch in enumerate(chunks):
                    sched.append((s, ci, tok0, ch))
                    tok0 += ch

            x_tiles = {}

            def emit_x(k):
                s, ci, tok0, ch = sched[k]
                xtd = xta if s == 0 else xtb
                t = xpool.tile([128, n_ct, ch], BF16, name=f"xt{k}", tag="xt")
                nc.sync.dma_start(out=t, in_=xtd[:, :, tok0 : tok0 + ch])
                x_tiles[k] = t

            # ---- startup-critical DMA order ----
            # x chunk-0 and the first W1a f-slice gate the first L1 f-tile
            # group; W1a streams in pieces sized so L1 chunk-0 never outruns
            # the DMA.  Everything after hides under compute (L2 lags L1 by a
            # chunk, so W2a hides under L1 of chunk 0).
            emit_x(0)
            w1a_cuts = [0, 256, 512, 1024, 1536, FH]
            nc.sync.dma_start(
                out=w1_sb[0][:, :, w1a_cuts[0] : w1a_cuts[1]],
                in_=w1a[:, :, w1a_cuts[0] : w1a_cuts[1]],
            )
            for s in range(2):
                nc.sync.dma_start(out=b_sb[:, s, :], in_=bt[s])
            for lo, hi in zip(w1a_cuts[1:], w1a_cuts[2:]):
                nc.sync.dma_start(
                    out=w1_sb[0][:, :, lo:hi], in_=w1a[:, :, lo:hi]
                )
            if len(sched) > 1:
                emit_x(1)
            nc.sync.dma_start(out=w2_sb[0], in_=w2a[:, :, :])
            if len(sched) > 2:
                emit_x(2)

            deferred_loads = [
                [lambda: nc.sync.dma_start(out=w1_sb[1], in_=w1b[:, :, :])],
                [lambda: nc.sync.dma_start(out=w2_sb[1], in_=w2b[:, :, :])],
            ]

            def do_l1(k):
                s, ci, tok0, ch = sched[k]
                xt = x_tiles[k]
                hts = []
                for f in range(n_ft):
                    ph = phpool.tile([128, ch], F32, name=f"ph{k}_{f}", tag="ph")
                    for c in range(n_ct):
                        nc.tensor.matmul(
                            ph,
                            lhsT=w1_sb[s][:, c, f * 128 : (f + 1) * 128],
                            rhs=xt[:, c, :],
                            start=(c == 0),
                            stop=(c == n_ct - 1),
                        )
                    ht = hpool.tile([128, ch], BF16, name=f"ht{k}_{f}", tag="ht")
                    nc.scalar.activation(
                        out=ht,
                        in_=ph,
                        func=mybir.ActivationFunctionType.Gelu,
                        bias=b_sb[:, s, f : f + 1],
                        scale=1.0,
                    )
                    hts.append(ht)
                return hts

            def do_l2(k, hts):
                s, ci, tok0, ch = sched[k]
                is_last = k == len(sched) - 1
                outd = outa if s == 0 else outb
                ot = opool.tile([128, n_cg, 512], BF16, name=f"ot{k}", tag="ot")
                for cg in range(n_cg):
                    po = popool.tile([128, 512], F32, name=f"po{k}_{cg}", tag="po")
                    for f in range(n_ft):
                        nc.tensor.matmul(
                            po[:, :ch],
                            lhsT=w2_sb[s][:, f, cg * 128 : (cg + 1) * 128],
                            rhs=hts[f],
                            start=(f == 0),
                            stop=(f == n_ft - 1),
                        )
                    nc.scalar.activation(
                        out=ot[:, cg, :ch],
                        in_=po[:, :ch],
                        func=mybir.ActivationFunctionType.Identity,
                        bias=b_sb[:, s, n_ft + cg : n_ft + cg + 1],
                        scale=1.0,
                    )
                    if is_last:
                        # stagger the last chunk's stores per c-group so the
                        # final flush after the last matmul is one small DMA
                        nc.sync.dma_start(
                            out=outd[:, cg, tok0 : tok0 + ch],
                            in_=ot[:, cg, :ch],
                        )
                    elif cg == n_cg // 2 - 1:
                        nc.sync.dma_start(
                            out=outd[:, : n_cg // 2, tok0 : tok0 + ch],
                            in_=ot[:, : n_cg // 2, :ch],
                        )
                if not is_last:
                    nc.sync.dma_start(
                        out=outd[:, n_cg // 2 :, tok0 : tok0 + ch],
                        in_=ot[:, n_cg // 2 :, :ch],
                    )

            prev = None
            prev_hts = None
            for k in range(len(sched)):
                if k + 3 < len(sched):
                    emit_x(k + 3)
                if k >= 1 and deferred_loads:
                    for emit in deferred_loads.pop(0):
                        emit()
                hts = do_l1(k)
                if prev is not None:
                    do_l2(prev, prev_hts)
                prev, prev_hts = k, hts
            do_l2(prev, prev_hts)
    nc.finalize()
    return nc


def pick_chunks(n: int) -> list[int]:
    """[512]*a + [exact tail] — matmul N needs no alignment."""
    n512 = n // 512
    rem = n - n512 * 512
    chunks = [512] * n512
    if rem > 0:
        chunks.append(rem)
    if not chunks:
        chunks = [1]
    return chunks


def _route(x2d: np.ndarray, Wg: np.ndarray):
    """fp32 gate identical in selection to the reference; returns per-expert
    token indices and renormalized top-2 weights."""
    logits = x2d @ Wg  # fp32 BLAS
    order = np.argsort(-logits, axis=1, kind="stable")
    top2 = order[:, :K]  # [N, 2]
    m = logits.max(axis=1, keepdims=True)
    p = np.exp(logits - m, dtype=np.float32)
    p /= p.sum(axis=1, keepdims=True)
    tw = np.take_along_axis(p, top2, axis=1)
    tw /= tw.sum(axis=1, keepdims=True)  # [N, 2] renormalized
    idxs, ws = [], []
    for e in range(E):
        sel = top2 == e  # [N, 2] bool, at most one True per row
        rows = np.where(sel.any(axis=1))[0]
        idxs.append(rows)
        ws.append(tw[rows][sel[rows]])
    return idxs, ws


def _pmajor(a: np.ndarray, ktiles: int) -> np.ndarray:
    """[ktiles*128, free] -> contiguous [128, ktiles, free]."""
    kt, rem = divmod(a.shape[0], 128)
    assert rem == 0 and kt == ktiles
    return np.ascontiguousarray(a.reshape(ktiles, 128, -1).transpose(1, 0, 2))


_LAST_RESULTS = {}  # stash for test harness introspection (exec time etc.)


def kernel(**inputs: np.ndarray) -> np.ndarray:
    x = np.asarray(inputs["x"], dtype=np.float32)
    Wg = np.asarray(inputs["Wg"], dtype=np.float32)
    W1 = np.asarray(inputs["W1"], dtype=np.float32)
    b1 = np.asarray(inputs["b1"], dtype=np.float32)
    W2 = np.asarray(inputs["W2"], dtype=np.float32)
    b2 = np.asarray(inputs["b2"], dtype=np.float32)

    B, T, Cx = x.shape
    assert Cx == C
    x2d = np.ascontiguousarray(x.reshape(-1, C))
    n_tok_total = x2d.shape[0]

    idxs, ws = _route(x2d, Wg)
    counts = np.array([len(i) for i in idxs])

    # Pair the largest expert with the smallest, 2nd largest with 2nd
    # smallest, etc.  Pair p runs on cores 2p (F-half 0) and 2p+1 (F-half 1).
    order = np.argsort(-counts, kind="stable")
    pairs = [(int(order[p]), int(order[E - 1 - p])) for p in range(E // 2)]
    nta = max(counts[a] for a, _ in pairs)
    ntb = max(counts[b] for _, b in pairs)
    chunks_a = pick_chunks(int(nta))
    chunks_b = pick_chunks(int(ntb))
    nta, ntb = sum(chunks_a), sum(chunks_b)

    w1h = W1.astype(ml_dtypes.bfloat16)  # [E, C, F]
    w2h = W2.astype(ml_dtypes.bfloat16)  # [E, F, C]

    def xt_for(e, ntok):
        xe = np.zeros((ntok, C), dtype=np.float32)
        xe[: counts[e]] = x2d[idxs[e]]
        return _pmajor(xe.T.astype(ml_dtypes.bfloat16), C // 128)

    xt_cache = {}
    for a, b_ in pairs:
        xt_cache[a] = xt_for(a, nta)
        xt_cache[b_] = xt_for(b_, ntb)

    in_maps = []
    for core in range(N_CORES):
        p, h = divmod(core, 2)
        ea, eb = pairs[p]
        fsl = slice(h * FH, (h + 1) * FH)
        bt = np.stack(
            [
                np.concatenate(
                    [
                        b1[e][fsl].reshape(FH // 128, 128).T,
                        b2[e].reshape(C // 128, 128).T * 0.5,
                    ],
                    axis=1,
                )
                for e in (ea, eb)
            ]
        ).astype(np.float32)
        in_maps.append(
            {
                "xta": xt_cache[ea],
                "xtb": xt_cache[eb],
                "w1a": _pmajor(np.ascontiguousarray(w1h[ea][:, fsl]), C // 128),
                "w1b": _pmajor(np.ascontiguousarray(w1h[eb][:, fsl]), C // 128),
                "w2a": _pmajor(np.ascontiguousarray(w2h[ea][fsl, :]), FH // 128),
                "w2b": _pmajor(np.ascontiguousarray(w2h[eb][fsl, :]), FH // 128),
                "bt": np.ascontiguousarray(bt),
            }
        )

    nc = build_nc(chunks_a, chunks_b)
    trace = os.environ.get("KERNEL_TRACE", "") == "1"
    res = run_bass_kernel_spmd(
        nc, in_maps, core_ids=list(range(N_CORES)), trace=trace
    )
    _LAST_RESULTS["bass_results"] = res
    if trace and res.exec_time_ns is not None:
        print(f"[kernel] HW exec time: {res.exec_time_ns} ns")

    out = np.zeros((n_tok_total, C), dtype=np.float32)
    for p, (ea, eb) in enumerate(pairs):
        for e, key in ((ea, "outa"), (eb, "outb")):
            n_e = counts[e]
            o0 = np.asarray(res.results[2 * p][key]).astype(np.float32)
            o1 = np.asarray(res.results[2 * p + 1][key]).astype(np.float32)
            oe = (o0 + o1).transpose(1, 0, 2).reshape(C, -1).T[:n_e]
            out[idxs[e]] += ws[e][:, None] * oe
    return out.reshape(B, T, C)


# revision 9
# speedup vs baseline: 1.0382x; 1.0032x over previous
"""Trainium2 Bass kernel for an 8-expert top-2 MoE layer (B=4, T=2048, C=1024,
F=4096), expert-parallel across 8 NeuronCores.

Strategy
--------
The reference module is a *dense* MoE: it runs every expert's FFN on every
token then combines with top-2 gate weights, so 6 of 8 expert outputs per
token are multiplied by zero.  We route instead: the host computes the gate in
fp32 (bf16 flips the selected expert set for ~17 near-tie tokens), assigns
each token to its two experts, the device runs each expert's FFN over just its
tokens, and the host scatter-adds the gate-weighted outputs.

Work layout: every expert's FFN is split into two F-halves; core 2p+h runs
F-half h of two expert "body" slots (A and B) plus one small "overflow" slot C.
An SPMD program pads every slot to the largest instance across cores, so slot
caps are chosen to minimize cap_A + cap_B + cap_C subject to the overflow
pieces (tokens beyond a body cap) fitting the 8 C-instances: for the observed
routing this gives ~4129 padded tokens/core vs 4204 for plain big-small expert
pairing (ideal balance is 4096).  Slot C reuses slot A's SBUF weight buffers —
its weights stream in after slot A's last L1 read.

On-device math per core and slot (expert e, F-half h):
    hT[f, t]   = sum_c W1[c, f] * xT[c, t]        (PE, bf16, fp32 acc)
    hT         = gelu_erf(hT + b1[f])             (ScalarE, fused bias)
    outT[c, t] = sum_{f in half} W2[f, c] * hT[f, t]   (PE, W2 stationary)
    outT       = outT + b2/2, cast bf16           (ScalarE Identity w/ bias)
L2 keeps W2 stationary and hT moving so the matmul free dim is the *exact*
chunk token count and the output lands transposed; the host transposes back.
L2 of chunk k is issued after L1 of chunk k+1 (software pipelining): L1 of the
first chunk hides the W2 DMA and L2 never waits on its own chunk's last gelu.

All large tensors use partition-major DRAM layouts ([128, ktiles, free]) so
each moves in O(1) dma_start calls — DMA *issue* costs ~0.8us each on the sync
queue, and per-tile DMAs made startup issue-bound.  A short spin of dummy
matmuls with no DMA dependency keeps the Tensor engine busy through the
initial fill so its clock (HAM pstate) is fully ramped at the first real MM.
"""

import math
import os
from itertools import combinations

import numpy as np
import ml_dtypes

import concourse.bass as bass
import concourse.mybir as mybir
import concourse.tile as tile
from concourse import bacc
from concourse.bass_utils import run_bass_kernel_spmd

C = 1024
F = 4096
FH = F // 2  # per-core F half
E = 8
K = 2
N_CORES = 8

BF16 = mybir.dt.bfloat16
F32 = mybir.dt.float32

N_CT = C // 128  # 8 contraction tiles for x @ W1
N_FT = FH // 128  # 16 F tiles per half
N_CG = C // 128  # 8 output c-tile groups for L2


def build_nc(chunks: list[tuple[list[int], int]]) -> bass.Bass:
    """Bass program over slots; chunks = [(chunk_list, weight_slot)] where
    weight_slot 0/1 selects the SBUF weight buffer (slot C reuses 0)."""
    nc = bacc.Bacc(None)

    n_slots = len(chunks)
    nts = [sum(cl) for cl, _ in chunks]
    xds = [
        nc.dram_tensor(f"xt{i}", [128, N_CT, nts[i]], BF16, kind="ExternalInput")
        for i in range(n_slots)
    ]
    w1ds = [
        nc.dram_tensor(f"w1{i}", [128, N_CT, FH], BF16, kind="ExternalInput")
        for i in range(n_slots)
    ]
    w2ds = [
        nc.dram_tensor(f"w2{i}", [128, N_FT, C], BF16, kind="ExternalInput")
        for i in range(n_slots)
    ]
    bt = nc.dram_tensor("bt", [n_slots, 128, N_FT + N_CG], F32, kind="ExternalInput")
    outds = [
        nc.dram_tensor(f"out{i}", [128, N_CG, nts[i]], BF16, kind="ExternalOutput")
        for i in range(n_slots)
    ]

    with tile.TileContext(nc) as tc:
        with (
            tc.tile_pool(name="wpool", bufs=1) as wpool,
            tc.tile_pool(name="xpool", bufs=3) as xpool,
            tc.tile_pool(name="hpool", bufs=2 * N_FT + 2) as hpool,
            tc.tile_pool(name="opool", bufs=2) as opool,
            tc.tile_pool(name="phpool", bufs=3, space="PSUM") as phpool,
            tc.tile_pool(name="popool", bufs=4, space="PSUM") as popool,
            tc.tile_pool(name="wupool", bufs=1, space="PSUM") as wupool,
        ):
            # PE warmup: dummy matmuls with no DMA dependency spin the Tensor
            # engine through the initial DMA fill so the clock (HAM pstate) is
            # ramped and never re-throttles before the first real matmul.
            dmy = wpool.tile([128, 512], BF16, name="dmy", tag="dmy")
            nc.vector.memset(dmy, 0.0)
            wups = wupool.tile([128, 512], F32, name="wups", tag="wups")
            for _ in range(12):
                nc.tensor.matmul(
                    wups, lhsT=dmy[:, :128], rhs=dmy, start=True, stop=True
                )

            w1_sb = {
                s: wpool.tile([128, N_CT, FH], BF16, name=f"w1sb{s}", tag=f"w1sb{s}")
                for s in range(2)
            }
            w2_sb = {
                s: wpool.tile([128, N_FT, C], BF16, name=f"w2sb{s}", tag=f"w2sb{s}")
                for s in range(2)
            }
            b_sb = wpool.tile(
                [128, n_slots, N_FT + N_CG], F32, name="bsb", tag="bsb"
            )

            # global chunk schedule: (slot, tok0, ch)
            sched = []
            for s, (cl, _) in enumerate(chunks):
                tok0 = 0
                for ch in cl:
                    sched.append((s, tok0, ch))
                    tok0 += ch

            x_tiles = {}

            def emit_x(k):
                s, tok0, ch = sched[k]
                t = xpool.tile([128, N_CT, ch], BF16, name=f"xt{k}", tag="xt")
                nc.sync.dma_start(out=t, in_=xds[s][:, :, tok0 : tok0 + ch])
                x_tiles[k] = t

            # ---- startup-critical DMA order ----
            # x chunk-0 and the first W1 f-slice gate the first L1 f-tile
            # group; W1 streams in pieces sized so L1 chunk-0 never outruns
            # the DMA.  W2 of slot 0 hides under L1 chunk 0 (L2 lags L1 by a
            # chunk).
            emit_x(0)
            w1_cuts = [0, 256, 512, 1024, 1536, FH]
            nc.sync.dma_start(
                out=w1_sb[0][:, :, : w1_cuts[1]], in_=w1ds[0][:, :, : w1_cuts[1]]
            )
            for s in range(n_slots):
                nc.sync.dma_start(out=b_sb[:, s, :], in_=bt[s])
            for lo, hi in zip(w1_cuts[1:], w1_cuts[2:]):
                nc.sync.dma_start(out=w1_sb[0][:, :, lo:hi], in_=w1ds[0][:, :, lo:hi])
            if len(sched) > 1:
                emit_x(1)
            nc.sync.dma_start(out=w2_sb[0], in_=w2ds[0][:, :, :])
            if len(sched) > 2:
                emit_x(2)

            # deferred weight loads: (emit at global chunk index, fn).  Slot C
            # (weight_slot 0 again) streams into slot A's buffers after slot
            # A's last L1/L2 reads; the tile framework sequences the WAR.
            slot_first_k = {}
            kk = 0
            for s, (cl, _) in enumerate(chunks):
                slot_first_k[s] = kk
                kk += len(cl)
            deferred = []
            for s in range(1, n_slots):
                ws = chunks[s][1]
                # slot s's weights: emit 2 chunks into the previous slot's run
                at_k = max(1, slot_first_k[s] - 3)
                deferred.append(
                    (at_k, lambda s=s, ws=ws: nc.sync.dma_start(
                        out=w1_sb[ws], in_=w1ds[s][:, :, :]))
                )
                deferred.append(
                    (at_k + 1, lambda s=s, ws=ws: nc.sync.dma_start(
                        out=w2_sb[ws], in_=w2ds[s][:, :, :]))
                )
            deferred.sort(key=lambda t: t[0])

            def do_l1(k):
                s, tok0, ch = sched[k]
                ws = chunks[s][1]
                xt = x_tiles[k]
                hts = []
                for f in range(N_FT):
                    ph = phpool.tile([128, ch], F32, name=f"ph{k}_{f}", tag="ph")
                    for c in range(N_CT):
                        nc.tensor.matmul(
                            ph,
                            lhsT=w1_sb[ws][:, c, f * 128 : (f + 1) * 128],
                            rhs=xt[:, c, :],
                            start=(c == 0),
                            stop=(c == N_CT - 1),
                        )
                    ht = hpool.tile([128, ch], BF16, name=f"ht{k}_{f}", tag="ht")
                    nc.scalar.activation(
                        out=ht,
                        in_=ph,
                        func=mybir.ActivationFunctionType.Gelu,
                        bias=b_sb[:, s, f : f + 1],
                        scale=1.0,
                    )
                    hts.append(ht)
                return hts

            def do_l2(k, hts):
                s, tok0, ch = sched[k]
                ws = chunks[s][1]
                is_last = k == len(sched) - 1
                outd = outds[s]
                ot = opool.tile([128, N_CG, 512], BF16, name=f"ot{k}", tag="ot")
                for cg in range(N_CG):
                    po = popool.tile([128, 512], F32, name=f"po{k}_{cg}", tag="po")
                    for f in range(N_FT):
                        nc.tensor.matmul(
                            po[:, :ch],
                            lhsT=w2_sb[ws][:, f, cg * 128 : (cg + 1) * 128],
                            rhs=hts[f],
                            start=(f == 0),
                            stop=(f == N_FT - 1),
                        )
                    nc.scalar.activation(
                        out=ot[:, cg, :ch],
                        in_=po[:, :ch],
                        func=mybir.ActivationFunctionType.Identity,
                        bias=b_sb[:, s, N_FT + cg : N_FT + cg + 1],
                        scale=1.0,
                    )
                    if is_last:
                        # stagger the last chunk's stores per c-group so the
                        # final flush after the last matmul is one small DMA
                        nc.sync.dma_start(
                            out=outd[:, cg, tok0 : tok0 + ch], in_=ot[:, cg, :ch]
                        )
                    elif cg == N_CG // 2 - 1:
                        nc.sync.dma_start(
                            out=outd[:, : N_CG // 2, tok0 : tok0 + ch],
                            in_=ot[:, : N_CG // 2, :ch],
                        )
                if not is_last:
                    nc.sync.dma_start(
                        out=outd[:, N_CG // 2 :, tok0 : tok0 + ch],
                        in_=ot[:, N_CG // 2 :, :ch],
                    )

            prev = None
            prev_hts = None
            for k in range(len(sched)):
                if k + 3 < len(sched):
                    emit_x(k + 3)
                while deferred and deferred[0][0] <= k:
                    deferred.pop(0)[1]()
                hts = do_l1(k)
                if prev is not None:
                    do_l2(prev, prev_hts)
                prev, prev_hts = k, hts
            while deferred:
                deferred.pop(0)[1]()
            do_l2(prev, prev_hts)
    nc.finalize()
    return nc


def pick_chunks(n: int) -> list[int]:
    """[512]*a + [exact tail] — matmul N needs no alignment."""
    n512 = n // 512
    rem = n - n512 * 512
    chunks = [512] * n512
    if rem > 0:
        chunks.append(rem)
    if not chunks:
        chunks = [1]
    return chunks


def plan_schedule(counts: np.ndarray):
    """Choose body caps (sA, sB), class split, and overflow cap mu minimizing
    padded tokens/core, with overflow pieces fitting the 8 C-instances.

    Returns (clsA, sA, clsB, sB, mu, parts) where parts is a list of up to 8
    (expert, tok_off, length); parts apply to BOTH F-halves symmetrically, so
    a part at index i runs on cores 2*(i//2) + (i%2)... (assignment done by
    caller).  mu == 0 means no overflow slot is needed.
    """

    def min_mu(ovs):
        if not ovs:
            return 0
        lo, hi = max(1, math.ceil(sum(ovs) / 8)), 512
        best = None
        while lo <= hi:
            mid = (lo + hi) // 2
            if sum(math.ceil(o / mid) for o in ovs) <= 8:
                best, hi = mid, mid - 1
            else:
                lo = mid + 1
        return best  # None if infeasible

    def evaluate(A, B, sA, sB):
        ovs = []
        for e in A:
            if counts[e] > sA:
                ovs += [int(counts[e] - sA)] * 2
        for e in B:
            if counts[e] > sB:
                ovs += [int(counts[e] - sB)] * 2
        mu = min_mu(ovs)
        if mu is None:
            return None
        return sA + sB + mu, mu

    best = None
    vals = sorted({int(c) for c in counts})
    for A in combinations(range(E), E // 2):
        B = tuple(i for i in range(E) if i not in A)
        for sA in vals:
            for sB in vals:
                r = evaluate(A, B, sA, sB)
                if r and (best is None or r[0] < best[0]):
                    best = (r[0], A, B, sA, sB, r[1])
    # local refine around the best caps
    _, A, B, sA0, sB0, _ = best
    for sA in range(max(1, sA0 - 64), sA0 + 65):
        for sB in range(max(1, sB0 - 64), sB0 + 65):
            r = evaluate(A, B, sA, sB)
            if r and r[0] < best[0]:
                best = (r[0], A, B, sA, sB, r[1])
    _, A, B, sA, sB, mu = best

    parts = []  # (expert, off, len) — same split for both F-halves
    if mu > 0:
        for cls, cap in ((A, sA), (B, sB)):
            for e in cls:
                rem = int(counts[e]) - cap
                off = cap
                while rem > 0:
                    ln = min(mu, rem)
                    parts.append((e, off, ln))
                    off += ln
                    rem -= ln
    assert 2 * len(parts) <= 8
    return list(A), sA, list(B), sB, mu, parts


def _route(x2d: np.ndarray, Wg: np.ndarray):
    """fp32 gate identical in selection to the reference; returns per-expert
    token indices and renormalized top-2 weights."""
    logits = x2d @ Wg  # fp32 BLAS
    order = np.argsort(-logits, axis=1, kind="stable")
    top2 = order[:, :K]  # [N, 2]
    m = logits.max(axis=1, keepdims=True)
    p = np.exp(logits - m, dtype=np.float32)
    p /= p.sum(axis=1, keepdims=True)
    tw = np.take_along_axis(p, top2, axis=1)
    tw /= tw.sum(axis=1, keepdims=True)  # [N, 2] renormalized
    idxs, ws = [], []
    for e in range(E):
        sel = top2 == e  # [N, 2] bool, at most one True per row
        rows = np.where(sel.any(axis=1))[0]
        idxs.append(rows)
        ws.append(tw[rows][sel[rows]])
    return idxs, ws


def _pmajor(a: np.ndarray, ktiles: int) -> np.ndarray:
    """[ktiles*128, free] -> contiguous [128, ktiles, free]."""
    kt, rem = divmod(a.shape[0], 128)
    assert rem == 0 and kt == ktiles
    return np.ascontiguousarray(a.reshape(ktiles, 128, -1).transpose(1, 0, 2))


_LAST_RESULTS = {}  # stash for test harness introspection (exec time etc.)


def kernel(**inputs: np.ndarray) -> np.ndarray:
    x = np.asarray(inputs["x"], dtype=np.float32)
    Wg = np.asarray(inputs["Wg"], dtype=np.float32)
    W1 = np.asarray(inputs["W1"], dtype=np.float32)
    b1 = np.asarray(inputs["b1"], dtype=np.float32)
    W2 = np.asarray(inputs["W2"], dtype=np.float32)
    b2 = np.asarray(inputs["b2"], dtype=np.float32)

    B, T, Cx = x.shape
    assert Cx == C
    x2d = np.ascontiguousarray(x.reshape(-1, C))
    n_tok_total = x2d.shape[0]

    idxs, ws = _route(x2d, Wg)
    counts = np.array([len(i) for i in idxs])

    clsA, sA, clsB, sB, mu, parts = plan_schedule(counts)
    has_c = mu > 0
    chunks = [(pick_chunks(sA), 0), (pick_chunks(sB), 1)]
    if has_c:
        chunks.append((pick_chunks(mu), 0))
    nta, ntb = sum(chunks[0][0]), sum(chunks[1][0])

    w1h = W1.astype(ml_dtypes.bfloat16)  # [E, C, F]
    w2h = W2.astype(ml_dtypes.bfloat16)  # [E, F, C]

    def xt_for(e, ntok, off=0):
        xe = np.zeros((ntok, C), dtype=np.float32)
        n = min(int(counts[e]) - off, ntok)
        xe[:n] = x2d[idxs[e][off : off + n]]
        return _pmajor(xe.T.astype(ml_dtypes.bfloat16), N_CT)

    xt_cache = {}
    for e in clsA:
        xt_cache[e] = xt_for(e, nta)
    for e in clsB:
        xt_cache[e] = xt_for(e, ntb)

    def bias_row(e, fsl):
        return np.concatenate(
            [
                b1[e][fsl].reshape(N_FT, 128).T,
                b2[e].reshape(N_CG, 128).T * 0.5,
            ],
            axis=1,
        )

    # C-instance assignment: part i of the (fh=0, fh=1) pair goes to cores
    # (2i, 2i+1)?? — simpler: flatten (part, fh) pairs over cores in order.
    cparts = []  # per core: (expert, off, len, fh)
    if has_c:
        flat = [(e, off, ln, fh) for (e, off, ln) in parts for fh in (0, 1)]
        assert len(flat) <= N_CORES
        while len(flat) < N_CORES:
            flat.append((0, 0, 0, 0))
        cparts = flat

    in_maps = []
    for core in range(N_CORES):
        p, h = divmod(core, 2)
        ea, eb = clsA[p], clsB[p]
        fsl = slice(h * FH, (h + 1) * FH)
        bias_rows = [bias_row(ea, fsl), bias_row(eb, fsl)]
        im = {
            "xt0": xt_cache[ea],
            "xt1": xt_cache[eb],
            "w10": _pmajor(np.ascontiguousarray(w1h[ea][:, fsl]), N_CT),
            "w11": _pmajor(np.ascontiguousarray(w1h[eb][:, fsl]), N_CT),
            "w20": _pmajor(np.ascontiguousarray(w2h[ea][fsl, :]), N_FT),
            "w21": _pmajor(np.ascontiguousarray(w2h[eb][fsl, :]), N_FT),
        }
        if has_c:
            ec, off, ln, fh = cparts[core]
            cfsl = slice(fh * FH, (fh + 1) * FH)
            im["xt2"] = xt_for(ec, mu, off=off) if ln > 0 else xt_for(0, mu, off=0)
            im["w12"] = _pmajor(np.ascontiguousarray(w1h[ec][:, cfsl]), N_CT)
            im["w22"] = _pmajor(np.ascontiguousarray(w2h[ec][cfsl, :]), N_FT)
            bias_rows.append(bias_row(ec, cfsl))
        im["bt"] = np.ascontiguousarray(np.stack(bias_rows)).astype(np.float32)
        in_maps.append(im)

    nc = build_nc(chunks)
    trace = os.environ.get("KERNEL_TRACE", "") == "1"
    res = run_bass_kernel_spmd(
        nc, in_maps, core_ids=list(range(N_CORES)), trace=trace
    )
    _LAST_RESULTS["bass_results"] = res
    if trace and res.exec_time_ns is not None:
        print(f"[kernel] HW exec time: {res.exec_time_ns} ns")

    def full_ct(core, key):
        # [128, N_CG, nt] bf16 -> [C, nt] fp32
        a = np.asarray(res.results[core][key]).astype(np.float32)
        return a.transpose(1, 0, 2).reshape(C, -1)

    out = np.zeros((n_tok_total, C), dtype=np.float32)
    for p in range(E // 2):
        for e, key, cap in ((clsA[p], "out0", sA), (clsB[p], "out1", sB)):
            n_e = int(counts[e])
            oe = np.zeros((n_e, C), dtype=np.float32)
            body = min(n_e, cap)
            oe[:body] = (full_ct(2 * p, key) + full_ct(2 * p + 1, key)).T[:body]
            if has_c and n_e > cap:
                for core, (ec, off, ln, fh) in enumerate(cparts):
                    if ec == e and ln > 0:
                        oe[off : off + ln] += full_ct(core, "out2").T[:ln]
            out[idxs[e]] += ws[e][:, None] * oe
    return out.reshape(B, T, C)


# revision 14
# speedup vs baseline: 1.0403x; 1.0020x over previous
"""Trainium2 Bass kernel for an 8-expert top-2 MoE layer (B=4, T=2048, C=1024,
F=4096), expert-parallel across 8 NeuronCores.

Strategy
--------
The reference module is a *dense* MoE: it runs every expert's FFN on every
token then combines with top-2 gate weights, so 6 of 8 expert outputs per
token are multiplied by zero.  We route instead: the host computes the gate in
fp32 (bf16 flips the selected expert set for ~17 near-tie tokens), assigns
each token to its two experts, the device runs each expert's FFN over just its
tokens, and the host scatter-adds the gate-weighted outputs.

Work layout: every expert's FFN is split into two F-halves; core 2p+h runs
F-half h of two expert "body" slots (A and B) plus one small "overflow" slot C.
An SPMD program pads every slot to the largest instance across cores, so slot
caps are chosen to minimize cap_A + cap_B + cap_C subject to the overflow
pieces (tokens beyond a body cap) fitting the 8 C-instances: for the observed
routing this gives ~4129 padded tokens/core vs 4204 for plain big-small expert
pairing (ideal balance is 4096).  Slot C reuses slot A's SBUF weight buffers —
its weights stream in after slot A's last L1 read.

On-device math per core and slot (expert e, F-half h):
    hT[f, t]   = sum_c W1[c, f] * xT[c, t]        (PE, bf16, fp32 acc)
    hT         = gelu_erf(hT + b1[f])             (ScalarE, fused bias)
    outT[c, t] = sum_{f in half} W2[f, c] * hT[f, t]   (PE, W2 stationary)
    outT       = outT + b2/2, cast bf16           (ScalarE Identity w/ bias)
L2 keeps W2 stationary and hT moving so the matmul free dim is the *exact*
chunk token count and the output lands transposed; the host transposes back.
L2 of chunk k is issued after L1 of chunk k+1 (software pipelining): L1 of the
first chunk hides the W2 DMA and L2 never waits on its own chunk's last gelu.

All large tensors use partition-major DRAM layouts ([128, ktiles, free]) so
each moves in O(1) dma_start calls — DMA *issue* costs ~0.8us each on the sync
queue, and per-tile DMAs made startup issue-bound.  A short spin of dummy
matmuls with no DMA dependency keeps the Tensor engine busy through the
initial fill so its clock (HAM pstate) is fully ramped at the first real MM.
"""

import math
import os
from itertools import combinations

import numpy as np
import ml_dtypes

import concourse.bass as bass
import concourse.mybir as mybir
import concourse.tile as tile
from concourse import bacc
from concourse.bass_utils import run_bass_kernel_spmd

C = 1024
F = 4096
FH = F // 2  # per-core F half
E = 8
K = 2
N_CORES = 8

BF16 = mybir.dt.bfloat16
F32 = mybir.dt.float32

N_CT = C // 128  # 8 contraction tiles for x @ W1
N_FT = FH // 128  # 16 F tiles per half
N_CG = C // 128  # 8 output c-tile groups for L2


def build_nc(chunks: list[tuple[list[int], int]]) -> bass.Bass:
    """Bass program over slots; chunks = [(chunk_list, weight_slot)] where
    weight_slot 0/1 selects the SBUF weight buffer (slot C reuses 0)."""
    nc = bacc.Bacc(None)

    n_slots = len(chunks)
    nts = [sum(cl) for cl, _ in chunks]
    xds = [
        nc.dram_tensor(f"xt{i}", [128, N_CT, nts[i]], BF16, kind="ExternalInput")
        for i in range(n_slots)
    ]
    w1ds = [
        nc.dram_tensor(f"w1{i}", [128, N_CT, FH], BF16, kind="ExternalInput")
        for i in range(n_slots)
    ]
    w2ds = [
        nc.dram_tensor(f"w2{i}", [128, N_FT, C], BF16, kind="ExternalInput")
        for i in range(n_slots)
    ]
    bt = nc.dram_tensor("bt", [n_slots, 128, N_FT + N_CG], F32, kind="ExternalInput")
    outds = [
        nc.dram_tensor(f"out{i}", [128, N_CG, nts[i]], BF16, kind="ExternalOutput")
        for i in range(n_slots)
    ]

    with tile.TileContext(nc) as tc:
        with (
            tc.tile_pool(name="wpool", bufs=1) as wpool,
            tc.tile_pool(name="xpool", bufs=3) as xpool,
            tc.tile_pool(name="hpool", bufs=2 * N_FT + 2) as hpool,
            tc.tile_pool(name="opool", bufs=2) as opool,
            tc.tile_pool(name="phpool", bufs=4, space="PSUM") as phpool,
            tc.tile_pool(name="popool", bufs=4, space="PSUM") as popool,
        ):
            # PE warmup: dummy matmuls with no DMA dependency spin the Tensor
            # engine through the initial DMA fill so the clock (HAM pstate) is
            # ramped and never re-throttles before the first real matmul.
            dmy = wpool.tile([128, 512], BF16, name="dmy", tag="dmy")
            nc.vector.memset(dmy, 0.0)
            wups = popool.tile([128, 512], F32, name="wups", tag="po")
            for _ in range(18):
                nc.tensor.matmul(
                    wups, lhsT=dmy[:, :128], rhs=dmy, start=True, stop=True
                )

            w1_sb = {
                s: wpool.tile([128, N_CT, FH], BF16, name=f"w1sb{s}", tag=f"w1sb{s}")
                for s in range(2)
            }
            w2_sb = {
                s: wpool.tile([128, N_FT, C], BF16, name=f"w2sb{s}", tag=f"w2sb{s}")
                for s in range(2)
            }
            b_sb = wpool.tile(
                [128, n_slots, N_FT + N_CG], F32, name="bsb", tag="bsb"
            )

            # global chunk schedule: (slot, tok0, ch).  Slot C (if present) is
            # inserted before slot B's LAST chunk so its scalar-paced tiny-
            # chunk overheads hide under the surrounding big chunks' L2 work.
            per_slot = []
            for s, (cl, _) in enumerate(chunks):
                tok0 = 0
                items = []
                for ch in cl:
                    items.append((s, tok0, ch))
                    tok0 += ch
                per_slot.append(items)
            sched = per_slot[0] + per_slot[1]
            if len(per_slot) > 2:
                sched = sched[:-1] + per_slot[2] + sched[-1:]

            x_tiles = {}

            def emit_x(k):
                s, tok0, ch = sched[k]
                t = xpool.tile([128, N_CT, ch], BF16, name=f"xt{k}", tag="xt")
                nc.sync.dma_start(out=t, in_=xds[s][:, :, tok0 : tok0 + ch])
                x_tiles[k] = t

            # ---- startup-critical DMA order ----
            # x chunk-0 and the first W1 f-slice gate the first L1 f-tile
            # group; W1 streams in pieces sized so L1 chunk-0 never outruns
            # the DMA.  W2 of slot 0 hides under L1 chunk 0 (L2 lags L1 by a
            # chunk).
            emit_x(0)
            w1_cuts = [0, 256, 512, 1024, 1536, FH]
            nc.sync.dma_start(
                out=w1_sb[0][:, :, : w1_cuts[1]], in_=w1ds[0][:, :, : w1_cuts[1]]
            )
            for s in range(n_slots):
                nc.sync.dma_start(out=b_sb[:, s, :], in_=bt[s])
            for lo, hi in zip(w1_cuts[1:], w1_cuts[2:]):
                nc.sync.dma_start(out=w1_sb[0][:, :, lo:hi], in_=w1ds[0][:, :, lo:hi])
            if len(sched) > 1:
                emit_x(1)
            nc.sync.dma_start(out=w2_sb[0], in_=w2ds[0][:, :, :])
            if len(sched) > 2:
                emit_x(2)

            # deferred weight loads: (emit at global chunk index, fn).  Slot C
            # (weight_slot 0 again) streams into slot A's buffers after slot
            # A's last L1/L2 reads; the tile framework sequences the WAR.
            slot_first_k = {}
            for k, (s, _, _) in enumerate(sched):
                if s not in slot_first_k:
                    slot_first_k[s] = k
            deferred = []
            for s in range(1, n_slots):
                ws = chunks[s][1]
                # slot s's weights: emit 2 chunks into the previous slot's run
                at_k = max(1, slot_first_k[s] - 3)
                deferred.append(
                    (at_k, lambda s=s, ws=ws: nc.sync.dma_start(
                        out=w1_sb[ws], in_=w1ds[s][:, :, :]))
                )
                deferred.append(
                    (at_k + 1, lambda s=s, ws=ws: nc.sync.dma_start(
                        out=w2_sb[ws], in_=w2ds[s][:, :, :]))
                )
            deferred.sort(key=lambda t: t[0])

            def do_l1(k):
                s, tok0, ch = sched[k]
                ws = chunks[s][1]
                xt = x_tiles[k]
                # pack several f-tiles into one PSUM bank for small chunks so
                # the bank rotation never waits on the (fixed-cost) gelus
                pack = max(1, min(N_FT, 512 // ch)) if ch <= 256 else 1
                hts = []
                f = 0
                while f < N_FT:
                    g = min(pack, N_FT - f)
                    ph = phpool.tile([128, g * ch], F32, name=f"ph{k}_{f}", tag="ph")
                    for j in range(g):
                        for c in range(N_CT):
                            nc.tensor.matmul(
                                ph[:, j * ch : (j + 1) * ch],
                                lhsT=w1_sb[ws][:, c, (f + j) * 128 : (f + j + 1) * 128],
                                rhs=xt[:, c, :],
                                start=(c == 0),
                                stop=(c == N_CT - 1),
                            )
                    for j in range(g):
                        ht = hpool.tile(
                            [128, ch], BF16, name=f"ht{k}_{f + j}", tag="ht"
                        )
                        nc.scalar.activation(
                            out=ht,
                            in_=ph[:, j * ch : (j + 1) * ch],
                            func=mybir.ActivationFunctionType.Gelu,
                            bias=b_sb[:, s, f + j : f + j + 1],
                            scale=1.0,
                        )
                        hts.append(ht)
                    f += g
                return hts

            def do_l2(k, hts):
                s, tok0, ch = sched[k]
                ws = chunks[s][1]
                is_last = k == len(sched) - 1
                outd = outds[s]
                ot = opool.tile([128, N_CG, 512], BF16, name=f"ot{k}", tag="ot")
                for cg in range(N_CG):
                    po = popool.tile([128, 512], F32, name=f"po{k}_{cg}", tag="po")
                    for f in range(N_FT):
                        nc.tensor.matmul(
                            po[:, :ch],
                            lhsT=w2_sb[ws][:, f, cg * 128 : (cg + 1) * 128],
                            rhs=hts[f],
                            start=(f == 0),
                            stop=(f == N_FT - 1),
                        )
                    nc.scalar.activation(
                        out=ot[:, cg, :ch],
                        in_=po[:, :ch],
                        func=mybir.ActivationFunctionType.Identity,
                        bias=b_sb[:, s, N_FT + cg : N_FT + cg + 1],
                        scale=1.0,
                    )
                    if is_last and ch >= 256:
                        # stagger a big last chunk's stores per c-group so the
                        # final flush after the last matmul is one small DMA
                        nc.sync.dma_start(
                            out=outd[:, cg, tok0 : tok0 + ch], in_=ot[:, cg, :ch]
                        )
                    elif ch >= 256 and cg == N_CG // 2 - 1:
                        nc.sync.dma_start(
                            out=outd[:, : N_CG // 2, tok0 : tok0 + ch],
                            in_=ot[:, : N_CG // 2, :ch],
                        )
                if ch < 256:
                    # small chunk: one store (8 issue slots would out-cost it)
                    nc.sync.dma_start(
                        out=outd[:, :, tok0 : tok0 + ch], in_=ot[:, :, :ch]
                    )
                elif not is_last:
                    nc.sync.dma_start(
                        out=outd[:, N_CG // 2 :, tok0 : tok0 + ch],
                        in_=ot[:, N_CG // 2 :, :ch],
                    )

            prev = None
            prev_hts = None
            for k in range(len(sched)):
                if k + 3 < len(sched):
                    emit_x(k + 3)
                while deferred and deferred[0][0] <= k:
                    deferred.pop(0)[1]()
                hts = do_l1(k)
                if prev is not None:
                    do_l2(prev, prev_hts)
                prev, prev_hts = k, hts
            while deferred:
                deferred.pop(0)[1]()
            do_l2(prev, prev_hts)
    nc.finalize()
    return nc


def pick_chunks(n: int) -> list[int]:
    """[512]*a + [exact tail] — matmul N needs no alignment."""
    n512 = n // 512
    rem = n - n512 * 512
    chunks = [512] * n512
    if rem > 0:
        chunks.append(rem)
    if not chunks:
        chunks = [1]
    return chunks


def plan_schedule(counts: np.ndarray):
    """Choose body caps (sA, sB), class split, and overflow cap mu minimizing
    padded tokens/core, with overflow pieces fitting the 8 C-instances.

    Returns (clsA, sA, clsB, sB, mu, parts) where parts is a list of up to 8
    (expert, tok_off, length); parts apply to BOTH F-halves symmetrically, so
    a part at index i runs on cores 2*(i//2) + (i%2)... (assignment done by
    caller).  mu == 0 means no overflow slot is needed.
    """

    def min_mu(ovs):
        if not ovs:
            return 0
        lo, hi = max(1, math.ceil(sum(ovs) / 8)), 512
        best = None
        while lo <= hi:
            mid = (lo + hi) // 2
            if sum(math.ceil(o / mid) for o in ovs) <= 8:
                best, hi = mid, mid - 1
            else:
                lo = mid + 1
        return best  # None if infeasible

    def evaluate(A, B, sA, sB):
        ovs = []
        for e in A:
            if counts[e] > sA:
                ovs += [int(counts[e] - sA)] * 2
        for e in B:
            if counts[e] > sB:
                ovs += [int(counts[e] - sB)] * 2
        mu = min_mu(ovs)
        if mu is None:
            return None
        return sA + sB + mu, mu

    best = None
    vals = sorted({int(c) for c in counts})
    for A in combinations(range(E), E // 2):
        B = tuple(i for i in range(E) if i not in A)
        for sA in vals:
            for sB in vals:
                r = evaluate(A, B, sA, sB)
                if r and (best is None or r[0] < best[0]):
                    best = (r[0], A, B, sA, sB, r[1])
    # local refine around the best caps
    _, A, B, sA0, sB0, _ = best
    for sA in range(max(1, sA0 - 64), sA0 + 65):
        for sB in range(max(1, sB0 - 64), sB0 + 65):
            r = evaluate(A, B, sA, sB)
            if r and r[0] < best[0]:
                best = (r[0], A, B, sA, sB, r[1])
    _, A, B, sA, sB, mu = best

    parts = []  # (expert, off, len) — same split for both F-halves
    if mu > 0:
        for cls, cap in ((A, sA), (B, sB)):
            for e in cls:
                rem = int(counts[e]) - cap
                off = cap
                while rem > 0:
                    ln = min(mu, rem)
                    parts.append((e, off, ln))
                    off += ln
                    rem -= ln
    assert 2 * len(parts) <= 8
    return list(A), sA, list(B), sB, mu, parts


def _route(x2d: np.ndarray, Wg: np.ndarray):
    """fp32 gate identical in selection to the reference; returns per-expert
    token indices and renormalized top-2 weights."""
    logits = x2d @ Wg  # fp32 BLAS
    order = np.argsort(-logits, axis=1, kind="stable")
    top2 = order[:, :K]  # [N, 2]
    m = logits.max(axis=1, keepdims=True)
    p = np.exp(logits - m, dtype=np.float32)
    p /= p.sum(axis=1, keepdims=True)
    tw = np.take_along_axis(p, top2, axis=1)
    tw /= tw.sum(axis=1, keepdims=True)  # [N, 2] renormalized
    idxs, ws = [], []
    for e in range(E):
        sel = top2 == e  # [N, 2] bool, at most one True per row
        rows = np.where(sel.any(axis=1))[0]
        idxs.append(rows)
        ws.append(tw[rows][sel[rows]])
    return idxs, ws


def _pmajor(a: np.ndarray, ktiles: int) -> np.ndarray:
    """[ktiles*128, free] -> contiguous [128, ktiles, free]."""
    kt, rem = divmod(a.shape[0], 128)
    assert rem == 0 and kt == ktiles
    return np.ascontiguousarray(a.reshape(ktiles, 128, -1).transpose(1, 0, 2))


_LAST_RESULTS = {}  # stash for test harness introspection (exec time etc.)


def kernel(**inputs: np.ndarray) -> np.ndarray:
    x = np.asarray(inputs["x"], dtype=np.float32)
    Wg = np.asarray(inputs["Wg"], dtype=np.float32)
    W1 = np.asarray(inputs["W1"], dtype=np.float32)
    b1 = np.asarray(inputs["b1"], dtype=np.float32)
    W2 = np.asarray(inputs["W2"], dtype=np.float32)
    b2 = np.asarray(inputs["b2"], dtype=np.float32)

    B, T, Cx = x.shape
    assert Cx == C
    x2d = np.ascontiguousarray(x.reshape(-1, C))
    n_tok_total = x2d.shape[0]

    idxs, ws = _route(x2d, Wg)
    counts = np.array([len(i) for i in idxs])

    clsA, sA, clsB, sB, mu, parts = plan_schedule(counts)
    has_c = mu > 0
    chunks = [(pick_chunks(sA), 0), (pick_chunks(sB), 1)]
    if has_c:
        chunks.append((pick_chunks(mu), 0))
    nta, ntb = sum(chunks[0][0]), sum(chunks[1][0])

    w1h = W1.astype(ml_dtypes.bfloat16)  # [E, C, F]
    w2h = W2.astype(ml_dtypes.bfloat16)  # [E, F, C]

    def xt_for(e, ntok, off=0):
        xe = np.zeros((ntok, C), dtype=np.float32)
        n = min(int(counts[e]) - off, ntok)
        xe[:n] = x2d[idxs[e][off : off + n]]
        return _pmajor(xe.T.astype(ml_dtypes.bfloat16), N_CT)

    xt_cache = {}
    for e in clsA:
        xt_cache[e] = xt_for(e, nta)
    for e in clsB:
        xt_cache[e] = xt_for(e, ntb)

    def bias_row(e, fsl):
        return np.concatenate(
            [
                b1[e][fsl].reshape(N_FT, 128).T,
                b2[e].reshape(N_CG, 128).T * 0.5,
            ],
            axis=1,
        )

    # C-instance assignment: part i of the (fh=0, fh=1) pair goes to cores
    # (2i, 2i+1)?? — simpler: flatten (part, fh) pairs over cores in order.
    cparts = []  # per core: (expert, off, len, fh)
    if has_c:
        flat = [(e, off, ln, fh) for (e, off, ln) in parts for fh in (0, 1)]
        assert len(flat) <= N_CORES
        while len(flat) < N_CORES:
            flat.append((0, 0, 0, 0))
        cparts = flat

    in_maps = []
    for core in range(N_CORES):
        p, h = divmod(core, 2)
        ea, eb = clsA[p], clsB[p]
        fsl = slice(h * FH, (h + 1) * FH)
        bias_rows = [bias_row(ea, fsl), bias_row(eb, fsl)]
        im = {
            "xt0": xt_cache[ea],
            "xt1": xt_cache[eb],
            "w10": _pmajor(np.ascontiguousarray(w1h[ea][:, fsl]), N_CT),
            "w11": _pmajor(np.ascontiguousarray(w1h[eb][:, fsl]), N_CT),
            "w20": _pmajor(np.ascontiguousarray(w2h[ea][fsl, :]), N_FT),
            "w21": _pmajor(np.ascontiguousarray(w2h[eb][fsl, :]), N_FT),
        }
        if has_c:
            ec, off, ln, fh = cparts[core]
            cfsl = slice(fh * FH, (fh + 1) * FH)
            im["xt2"] = xt_for(ec, mu, off=off) if ln > 0 else xt_for(0, mu, off=0)
            im["w12"] = _pmajor(np.ascontiguousarray(w1h[ec][:, cfsl]), N_CT)
            im["w22"] = _pmajor(np.ascontiguousarray(w2h[ec][cfsl, :]), N_FT)
            bias_rows.append(bias_row(ec, cfsl))
        im["bt"] = np.ascontiguousarray(np.stack(bias_rows)).astype(np.float32)
        in_maps.append(im)

    nc = build_nc(chunks)
    trace = os.environ.get("KERNEL_TRACE", "") == "1"
    res = run_bass_kernel_spmd(
        nc, in_maps, core_ids=list(range(N_CORES)), trace=trace
    )
    _LAST_RESULTS["bass_results"] = res
    if trace and res.exec_time_ns is not None:
        print(f"[kernel] HW exec time: {res.exec_time_ns} ns")

    def full_ct(core, key):
        # [128, N_CG, nt] bf16 -> [C, nt] fp32
        a = np.asarray(res.results[core][key]).astype(np.float32)
        return a.transpose(1, 0, 2).reshape(C, -1)

    out = np.zeros((n_tok_total, C), dtype=np.float32)
    for p in range(E // 2):
        for e, key, cap in ((clsA[p], "out0", sA), (clsB[p], "out1", sB)):
            n_e = int(counts[e])
            oe = np.zeros((n_e, C), dtype=np.float32)
            body = min(n_e, cap)
            oe[:body] = (full_ct(2 * p, key) + full_ct(2 * p + 1, key)).T[:body]
            if has_c and n_e > cap:
                for core, (ec, off, ln, fh) in enumerate(cparts):
                    if ec == e and ln > 0:
                        oe[off : off + ln] += full_ct(core, "out2").T[:ln]
            out[idxs[e]] += ws[e][:, None] * oe
    return out.reshape(B, T, C)


# revision 15
# speedup vs baseline: 1.0496x; 1.0089x over previous
"""Trainium2 Bass kernel for an 8-expert top-2 MoE layer (B=4, T=2048, C=1024,
F=4096), expert-parallel across 8 NeuronCores.

Strategy
--------
The reference module is a *dense* MoE: it runs every expert's FFN on every
token then combines with top-2 gate weights, so 6 of 8 expert outputs per
token are multiplied by zero.  We route instead: the host computes the gate in
fp32 (bf16 flips the selected expert set for ~17 near-tie tokens), assigns
each token to its two experts, the device runs each expert's FFN over just its
tokens, and the host scatter-adds the gate-weighted outputs.

Work layout: every expert's FFN is split into two F-halves; core 2p+h runs
F-half h of two expert "body" slots (A and B) plus one small "overflow" slot C.
An SPMD program pads every slot to the largest instance across cores, so slot
caps are chosen to minimize cap_A + cap_B + cap_C subject to the overflow
pieces (tokens beyond a body cap) fitting the 8 C-instances: for the observed
routing this gives ~4129 padded tokens/core vs 4204 for plain big-small expert
pairing (ideal balance is 4096).  Slot C reuses slot A's SBUF weight buffers —
its weights stream in after slot A's last L1 read.

On-device math per core and slot (expert e, F-half h):
    hT[f, t]   = sum_c W1[c, f] * xT[c, t]        (PE, bf16, fp32 acc)
    hT         = gelu_erf(hT + b1[f])             (ScalarE, fused bias)
    outT[c, t] = sum_{f in half} W2[f, c] * hT[f, t]   (PE, W2 stationary)
    outT       = outT + b2/2, cast bf16           (ScalarE Identity w/ bias)
L2 keeps W2 stationary and hT moving so the matmul free dim is the *exact*
chunk token count and the output lands transposed; the host transposes back.
L2 of chunk k is issued after L1 of chunk k+1 (software pipelining): L1 of the
first chunk hides the W2 DMA and L2 never waits on its own chunk's last gelu.

All large tensors use partition-major DRAM layouts ([128, ktiles, free]) so
each moves in O(1) dma_start calls — DMA *issue* costs ~0.8us each on the sync
queue, and per-tile DMAs made startup issue-bound.  A short spin of dummy
matmuls with no DMA dependency keeps the Tensor engine busy through the
initial fill so its clock (HAM pstate) is fully ramped at the first real MM.
"""

import math
import os
from itertools import combinations

import numpy as np
import ml_dtypes

import concourse.bass as bass
import concourse.mybir as mybir
import concourse.tile as tile
from concourse import bacc
from concourse.bass_utils import run_bass_kernel_spmd

C = 1024
F = 4096
FH = F // 2  # per-core F half
E = 8
K = 2
N_CORES = 8

BF16 = mybir.dt.bfloat16
F32 = mybir.dt.float32

N_CT = C // 128  # 8 contraction tiles for x @ W1
N_FT = FH // 128  # 16 F tiles per half
N_CG = C // 128  # 8 output c-tile groups for L2


def build_nc(chunks: list[tuple[list[int], int]]) -> bass.Bass:
    """Bass program over slots; chunks = [(chunk_list, weight_slot)] where
    weight_slot 0/1 selects the SBUF weight buffer (slot C reuses 0)."""
    nc = bacc.Bacc(None)

    n_slots = len(chunks)
    nts = [sum(cl) for cl, _ in chunks]
    xds = [
        nc.dram_tensor(f"xt{i}", [128, N_CT, nts[i]], BF16, kind="ExternalInput")
        for i in range(n_slots)
    ]
    w1ds = [
        nc.dram_tensor(f"w1{i}", [128, N_CT, FH], BF16, kind="ExternalInput")
        for i in range(n_slots)
    ]
    w2ds = [
        nc.dram_tensor(f"w2{i}", [128, N_FT, C], BF16, kind="ExternalInput")
        for i in range(n_slots)
    ]
    bt = nc.dram_tensor("bt", [n_slots, 128, N_FT + N_CG], F32, kind="ExternalInput")
    outds = [
        nc.dram_tensor(f"out{i}", [128, N_CG, nts[i]], BF16, kind="ExternalOutput")
        for i in range(n_slots)
    ]

    with tile.TileContext(nc) as tc:
        with (
            tc.tile_pool(name="wpool", bufs=1) as wpool,
            tc.tile_pool(name="xpool", bufs=3) as xpool,
            tc.tile_pool(name="hpool", bufs=2 * N_FT + 2) as hpool,
            tc.tile_pool(name="opool", bufs=2) as opool,
            tc.tile_pool(name="phpool", bufs=4, space="PSUM") as phpool,
            tc.tile_pool(name="popool", bufs=4, space="PSUM") as popool,
        ):
            # PE warmup: dummy matmuls with no DMA dependency spin the Tensor
            # engine through the initial DMA fill so the clock (HAM pstate) is
            # ramped and never re-throttles before the first real matmul.
            dmy = wpool.tile([128, 512], BF16, name="dmy", tag="dmy")
            nc.vector.memset(dmy, 0.0)
            wups = popool.tile([128, 512], F32, name="wups", tag="po")
            for _ in range(18):
                nc.tensor.matmul(
                    wups, lhsT=dmy[:, :128], rhs=dmy, start=True, stop=True
                )

            w1_sb = {
                s: wpool.tile([128, N_CT, FH], BF16, name=f"w1sb{s}", tag=f"w1sb{s}")
                for s in range(2)
            }
            w2_sb = {
                s: wpool.tile([128, N_FT, C], BF16, name=f"w2sb{s}", tag=f"w2sb{s}")
                for s in range(2)
            }
            b_sb = wpool.tile(
                [128, n_slots, N_FT + N_CG], F32, name="bsb", tag="bsb"
            )

            # global chunk schedule: (slot, tok0, ch).  Slot C (if present) is
            # inserted before slot B's LAST chunk so its scalar-paced tiny-
            # chunk overheads hide under the surrounding big chunks' L2 work.
            per_slot = []
            for s, (cl, _) in enumerate(chunks):
                tok0 = 0
                items = []
                for ch in cl:
                    items.append((s, tok0, ch))
                    tok0 += ch
                per_slot.append(items)
            b_items = per_slot[1]
            if len(b_items) >= 2 and b_items[-1][2] < 256:
                # keep a BIG chunk last: its L2 covers the small chunks'
                # store flushes so the kernel tail is one staggered store
                b_items = b_items[:-2] + [b_items[-1], b_items[-2]]
            sched = per_slot[0] + b_items
            if len(per_slot) > 2:
                sched = sched[:-1] + per_slot[2] + sched[-1:]

            x_tiles = {}

            def emit_x(k):
                s, tok0, ch = sched[k]
                t = xpool.tile([128, N_CT, ch], BF16, name=f"xt{k}", tag="xt")
                nc.sync.dma_start(out=t, in_=xds[s][:, :, tok0 : tok0 + ch])
                x_tiles[k] = t

            # ---- startup-critical DMA order ----
            # x chunk-0 and the first W1 f-slice gate the first L1 f-tile
            # group; W1 streams in pieces sized so L1 chunk-0 never outruns
            # the DMA.  W2 of slot 0 hides under L1 chunk 0 (L2 lags L1 by a
            # chunk).
            emit_x(0)
            w1_cuts = [0, 256, 512, 1024, 1536, FH]
            nc.sync.dma_start(
                out=w1_sb[0][:, :, : w1_cuts[1]], in_=w1ds[0][:, :, : w1_cuts[1]]
            )
            for s in range(n_slots):
                nc.sync.dma_start(out=b_sb[:, s, :], in_=bt[s])
            for lo, hi in zip(w1_cuts[1:], w1_cuts[2:]):
                nc.sync.dma_start(out=w1_sb[0][:, :, lo:hi], in_=w1ds[0][:, :, lo:hi])
            if len(sched) > 1:
                emit_x(1)
            nc.sync.dma_start(out=w2_sb[0], in_=w2ds[0][:, :, :])
            if len(sched) > 2:
                emit_x(2)

            # deferred weight loads: (emit at global chunk index, fn).  Slot C
            # (weight_slot 0 again) streams into slot A's buffers after slot
            # A's last L1/L2 reads; the tile framework sequences the WAR.
            slot_first_k = {}
            for k, (s, _, _) in enumerate(sched):
                if s not in slot_first_k:
                    slot_first_k[s] = k
            deferred = []
            for s in range(1, n_slots):
                ws = chunks[s][1]
                # slot s's weights: emit 2 chunks into the previous slot's run
                at_k = max(1, slot_first_k[s] - 3)
                deferred.append(
                    (at_k, lambda s=s, ws=ws: nc.sync.dma_start(
                        out=w1_sb[ws], in_=w1ds[s][:, :, :]))
                )
                deferred.append(
                    (at_k + 1, lambda s=s, ws=ws: nc.sync.dma_start(
                        out=w2_sb[ws], in_=w2ds[s][:, :, :]))
                )
            deferred.sort(key=lambda t: t[0])

            def do_l1(k):
                s, tok0, ch = sched[k]
                ws = chunks[s][1]
                xt = x_tiles[k]
                # pack several f-tiles into one PSUM bank for small chunks so
                # the bank rotation never waits on the (fixed-cost) gelus
                pack = max(1, min(N_FT, 512 // ch)) if ch <= 256 else 1
                hts = []
                f = 0
                while f < N_FT:
                    g = min(pack, N_FT - f)
                    ph = phpool.tile([128, g * ch], F32, name=f"ph{k}_{f}", tag="ph")
                    for j in range(g):
                        for c in range(N_CT):
                            nc.tensor.matmul(
                                ph[:, j * ch : (j + 1) * ch],
                                lhsT=w1_sb[ws][:, c, (f + j) * 128 : (f + j + 1) * 128],
                                rhs=xt[:, c, :],
                                start=(c == 0),
                                stop=(c == N_CT - 1),
                            )
                    for j in range(g):
                        ht = hpool.tile(
                            [128, ch], BF16, name=f"ht{k}_{f + j}", tag="ht"
                        )
                        nc.scalar.activation(
                            out=ht,
                            in_=ph[:, j * ch : (j + 1) * ch],
                            func=mybir.ActivationFunctionType.Gelu,
                            bias=b_sb[:, s, f + j : f + j + 1],
                            scale=1.0,
                        )
                        hts.append(ht)
                    f += g
                return hts

            def do_l2(k, hts):
                s, tok0, ch = sched[k]
                ws = chunks[s][1]
                is_last = k == len(sched) - 1
                outd = outds[s]
                ot = opool.tile([128, N_CG, 512], BF16, name=f"ot{k}", tag="ot")
                for cg in range(N_CG):
                    po = popool.tile([128, 512], F32, name=f"po{k}_{cg}", tag="po")
                    for f in range(N_FT):
                        nc.tensor.matmul(
                            po[:, :ch],
                            lhsT=w2_sb[ws][:, f, cg * 128 : (cg + 1) * 128],
                            rhs=hts[f],
                            start=(f == 0),
                            stop=(f == N_FT - 1),
                        )
                    nc.scalar.activation(
                        out=ot[:, cg, :ch],
                        in_=po[:, :ch],
                        func=mybir.ActivationFunctionType.Identity,
                        bias=b_sb[:, s, N_FT + cg : N_FT + cg + 1],
                        scale=1.0,
                    )
                    if is_last and ch >= 256:
                        # stagger a big last chunk's stores per c-group so the
                        # final flush after the last matmul is one small DMA
                        nc.sync.dma_start(
                            out=outd[:, cg, tok0 : tok0 + ch], in_=ot[:, cg, :ch]
                        )
                    elif ch >= 256 and cg == N_CG // 2 - 1:
                        nc.sync.dma_start(
                            out=outd[:, : N_CG // 2, tok0 : tok0 + ch],
                            in_=ot[:, : N_CG // 2, :ch],
                        )
                if ch < 256:
                    # small chunk: one store (8 issue slots would out-cost it)
                    nc.sync.dma_start(
                        out=outd[:, :, tok0 : tok0 + ch], in_=ot[:, :, :ch]
                    )
                elif not is_last:
                    nc.sync.dma_start(
                        out=outd[:, N_CG // 2 :, tok0 : tok0 + ch],
                        in_=ot[:, N_CG // 2 :, :ch],
                    )

            prev = None
            prev_hts = None
            for k in range(len(sched)):
                if k + 3 < len(sched):
                    emit_x(k + 3)
                while deferred and deferred[0][0] <= k:
                    deferred.pop(0)[1]()
                hts = do_l1(k)
                if prev is not None:
                    do_l2(prev, prev_hts)
                prev, prev_hts = k, hts
            while deferred:
                deferred.pop(0)[1]()
            do_l2(prev, prev_hts)
    nc.finalize()
    return nc


def pick_chunks(n: int) -> list[int]:
    """[512]*a + [exact tail] — matmul N needs no alignment."""
    n512 = n // 512
    rem = n - n512 * 512
    chunks = [512] * n512
    if rem > 0:
        chunks.append(rem)
    if not chunks:
        chunks = [1]
    return chunks


def plan_schedule(counts: np.ndarray):
    """Choose body caps (sA, sB), class split, and overflow cap mu minimizing
    padded tokens/core, with overflow pieces fitting the 8 C-instances.

    Returns (clsA, sA, clsB, sB, mu, parts) where parts is a list of up to 8
    (expert, tok_off, length); parts apply to BOTH F-halves symmetrically, so
    a part at index i runs on cores 2*(i//2) + (i%2)... (assignment done by
    caller).  mu == 0 means no overflow slot is needed.
    """

    def min_mu(ovs):
        if not ovs:
            return 0
        lo, hi = max(1, math.ceil(sum(ovs) / 8)), 512
        best = None
        while lo <= hi:
            mid = (lo + hi) // 2
            if sum(math.ceil(o / mid) for o in ovs) <= 8:
                best, hi = mid, mid - 1
            else:
                lo = mid + 1
        return best  # None if infeasible

    def evaluate(A, B, sA, sB):
        ovs = []
        for e in A:
            if counts[e] > sA:
                ovs += [int(counts[e] - sA)] * 2
        for e in B:
            if counts[e] > sB:
                ovs += [int(counts[e] - sB)] * 2
        mu = min_mu(ovs)
        if mu is None:
            return None
        return sA + sB + mu, mu

    best = None
    vals = sorted({int(c) for c in counts})
    for A in combinations(range(E), E // 2):
        B = tuple(i for i in range(E) if i not in A)
        for sA in vals:
            for sB in vals:
                r = evaluate(A, B, sA, sB)
                if r and (best is None or r[0] < best[0]):
                    best = (r[0], A, B, sA, sB, r[1])
    # local refine around the best caps
    _, A, B, sA0, sB0, _ = best
    for sA in range(max(1, sA0 - 64), sA0 + 65):
        for sB in range(max(1, sB0 - 64), sB0 + 65):
            r = evaluate(A, B, sA, sB)
            if r and r[0] < best[0]:
                best = (r[0], A, B, sA, sB, r[1])
    _, A, B, sA, sB, mu = best

    parts = []  # (expert, off, len) — same split for both F-halves
    if mu > 0:
        for cls, cap in ((A, sA), (B, sB)):
            for e in cls:
                rem = int(counts[e]) - cap
                off = cap
                while rem > 0:
                    ln = min(mu, rem)
                    parts.append((e, off, ln))
                    off += ln
                    rem -= ln
    assert 2 * len(parts) <= 8
    return list(A), sA, list(B), sB, mu, parts


def _route(x2d: np.ndarray, Wg: np.ndarray):
    """fp32 gate identical in selection to the reference; returns per-expert
    token indices and renormalized top-2 weights."""
    logits = x2d @ Wg  # fp32 BLAS
    order = np.argsort(-logits, axis=1, kind="stable")
    top2 = order[:, :K]  # [N, 2]
    m = logits.max(axis=1, keepdims=True)
    p = np.exp(logits - m, dtype=np.float32)
    p /= p.sum(axis=1, keepdims=True)
    tw = np.take_along_axis(p, top2, axis=1)
    tw /= tw.sum(axis=1, keepdims=True)  # [N, 2] renormalized
    idxs, ws = [], []
    for e in range(E):
        sel = top2 == e  # [N, 2] bool, at most one True per row
        rows = np.where(sel.any(axis=1))[0]
        idxs.append(rows)
        ws.append(tw[rows][sel[rows]])
    return idxs, ws


def _pmajor(a: np.ndarray, ktiles: int) -> np.ndarray:
    """[ktiles*128, free] -> contiguous [128, ktiles, free]."""
    kt, rem = divmod(a.shape[0], 128)
    assert rem == 0 and kt == ktiles
    return np.ascontiguousarray(a.reshape(ktiles, 128, -1).transpose(1, 0, 2))


_LAST_RESULTS = {}  # stash for test harness introspection (exec time etc.)


def kernel(**inputs: np.ndarray) -> np.ndarray:
    x = np.asarray(inputs["x"], dtype=np.float32)
    Wg = np.asarray(inputs["Wg"], dtype=np.float32)
    W1 = np.asarray(inputs["W1"], dtype=np.float32)
    b1 = np.asarray(inputs["b1"], dtype=np.float32)
    W2 = np.asarray(inputs["W2"], dtype=np.float32)
    b2 = np.asarray(inputs["b2"], dtype=np.float32)

    B, T, Cx = x.shape
    assert Cx == C
    x2d = np.ascontiguousarray(x.reshape(-1, C))
    n_tok_total = x2d.shape[0]

    idxs, ws = _route(x2d, Wg)
    counts = np.array([len(i) for i in idxs])

    clsA, sA, clsB, sB, mu, parts = plan_schedule(counts)
    has_c = mu > 0
    chunks = [(pick_chunks(sA), 0), (pick_chunks(sB), 1)]
    if has_c:
        chunks.append((pick_chunks(mu), 0))
    nta, ntb = sum(chunks[0][0]), sum(chunks[1][0])

    w1h = W1.astype(ml_dtypes.bfloat16)  # [E, C, F]
    w2h = W2.astype(ml_dtypes.bfloat16)  # [E, F, C]

    def xt_for(e, ntok, off=0):
        xe = np.zeros((ntok, C), dtype=np.float32)
        n = min(int(counts[e]) - off, ntok)
        xe[:n] = x2d[idxs[e][off : off + n]]
        return _pmajor(xe.T.astype(ml_dtypes.bfloat16), N_CT)

    xt_cache = {}
    for e in clsA:
        xt_cache[e] = xt_for(e, nta)
    for e in clsB:
        xt_cache[e] = xt_for(e, ntb)

    def bias_row(e, fsl):
        return np.concatenate(
            [
                b1[e][fsl].reshape(N_FT, 128).T,
                b2[e].reshape(N_CG, 128).T * 0.5,
            ],
            axis=1,
        )

    # C-instance assignment: part i of the (fh=0, fh=1) pair goes to cores
    # (2i, 2i+1)?? — simpler: flatten (part, fh) pairs over cores in order.
    cparts = []  # per core: (expert, off, len, fh)
    if has_c:
        flat = [(e, off, ln, fh) for (e, off, ln) in parts for fh in (0, 1)]
        assert len(flat) <= N_CORES
        while len(flat) < N_CORES:
            flat.append((0, 0, 0, 0))
        cparts = flat

    in_maps = []
    for core in range(N_CORES):
        p, h = divmod(core, 2)
        ea, eb = clsA[p], clsB[p]
        fsl = slice(h * FH, (h + 1) * FH)
        bias_rows = [bias_row(ea, fsl), bias_row(eb, fsl)]
        im = {
            "xt0": xt_cache[ea],
            "xt1": xt_cache[eb],
            "w10": _pmajor(np.ascontiguousarray(w1h[ea][:, fsl]), N_CT),
            "w11": _pmajor(np.ascontiguousarray(w1h[eb][:, fsl]), N_CT),
            "w20": _pmajor(np.ascontiguousarray(w2h[ea][fsl, :]), N_FT),
            "w21": _pmajor(np.ascontiguousarray(w2h[eb][fsl, :]), N_FT),
        }
        if has_c:
            ec, off, ln, fh = cparts[core]
            cfsl = slice(fh * FH, (fh + 1) * FH)
            im["xt2"] = xt_for(ec, mu, off=off) if ln > 0 else xt_for(0, mu, off=0)
            im["w12"] = _pmajor(np.ascontiguousarray(w1h[ec][:, cfsl]), N_CT)
            im["w22"] = _pmajor(np.ascontiguousarray(w2h[ec][cfsl, :]), N_FT)
            bias_rows.append(bias_row(ec, cfsl))
        im["bt"] = np.ascontiguousarray(np.stack(bias_rows)).astype(np.float32)
        in_maps.append(im)

    nc = build_nc(chunks)
    trace = os.environ.get("KERNEL_TRACE", "") == "1"
    res = run_bass_kernel_spmd(
        nc, in_maps, core_ids=list(range(N_CORES)), trace=trace
    )
    _LAST_RESULTS["bass_results"] = res
    if trace and res.exec_time_ns is not None:
        print(f"[kernel] HW exec time: {res.exec_time_ns} ns")

    def full_ct(core, key):
        # [128, N_CG, nt] bf16 -> [C, nt] fp32
        a = np.asarray(res.results[core][key]).astype(np.float32)
        return a.transpose(1, 0, 2).reshape(C, -1)

    out = np.zeros((n_tok_total, C), dtype=np.float32)
    for p in range(E // 2):
        for e, key, cap in ((clsA[p], "out0", sA), (clsB[p], "out1", sB)):
            n_e = int(counts[e])
            oe = np.zeros((n_e, C), dtype=np.float32)
            body = min(n_e, cap)
            oe[:body] = (full_ct(2 * p, key) + full_ct(2 * p + 1, key)).T[:body]
            if has_c and n_e > cap:
                for core, (ec, off, ln, fh) in enumerate(cparts):
                    if ec == e and ln > 0:
                        oe[off : off + ln] += full_ct(core, "out2").T[:ln]
            out[idxs[e]] += ws[e][:, None] * oe
    return out.reshape(B, T, C)
